# revision 42
# baseline (speedup 1.0000x reference)
"""TENER-style MultiHeadedAttention TRN2 kernel (8 NeuronCores, SPMD).

Sharding (tensor-parallel over heads x data-parallel over batch):
core c handles batch b = c//4 and the 4 heads [4*(c%4), 4*(c%4)+4),
over ALL 1024 query rows. Wq/Wv are split column-wise by head, Wo
row-wise; each core emits a PARTIAL output [S, D] and the host gather
sums the 4 partials per batch (the Wo all-reduce).

Key math: the TENER relative-position term after the shift trick is
  rel[s, j] = (q_s + v_bias_h) . pos[j - s]
and pos rows are sinusoids, so by angle addition the whole score is ONE
128-deep contraction per head:
  scores[j, s] = [k_j ; sin(w j) ; cos(w j)] . [q_s ; a_sin(s) ; a_cos(s)]
  a_sin = qv_sin*cos(w s) + qv_cos*sin(w s)
  a_cos = qv_cos*cos(w s) - qv_sin*sin(w s)

The swapped-q operand of the rotation (qv_cos paired with sin rows and
vice versa) is built on-chip: evict qp PSUM->SBUF (ACT), multiply by a
128x128 permutation matrix on the PE (one matmul per ft/half) instead
of a second full projection chain.

All inputs are HOST-PACKED into their final SBUF layouts so every load
is one contiguous DMA with multi-KB packets: the dynamic DMA queues
share one engine round-robin by packet, so small-element rearranging
DMAs starve behind large ones. Loads spread over three queues (sync q1
+ scalar q10 HW-DGE, gpsimd q0 SW-DGE).

All q/k-path matmuls run as float32r (full PE rate at free-dim >= 256).
The v projection runs in bf16. Softmax denominators come free via a
ones-column per head appended to v; normalization runs one
reciprocal (DVE, straight off PSUM) + partition-broadcast (gpsimd) +
multiply (DVE) per head, overlapped under the next head's attnv.
"""

import math
import os
import sys

sys.path.insert(0, "/opt/trn_rl_repo")

import numpy as np

B, S, D = 2, 1024, 1024
H, HD = 16, 64          # global heads, head_dim
HL = 4                  # local heads per core
HALF = 32               # sin/cos half of head_dim
NC_ = 8                 # cores
JT = S // 128            # 8 key tiles
CT = D // 128            # 8 contraction tiles
TB = 2 * S + 4           # tabs: coss | sinsw | vb cols(2) | vb_sw cols(2)

_cache: dict = {}


def _rne_fp32r(a):
    """Round fp32 -> fp32r (1s+8e+11m) with round-to-nearest-even."""
    u = np.ascontiguousarray(a, dtype=np.float32).view(np.uint32)
    lsb = (u >> np.uint32(12)) & np.uint32(1)
    return ((u + np.uint32(0x7FF) + lsb) & np.uint32(0xFFFFF000)).view(np.float32)


def _pack_cpn(m, p=128):
    """[C*p, N] -> [p, C*N] (SBUF layout: partition-major, c-blocks along free)."""
    cp, n = m.shape
    c = cp // p
    return np.ascontiguousarray(
        m.reshape(c, p, n).transpose(1, 0, 2).reshape(p, c * n))


def _build_nc(has_bq: bool, has_bo: bool):
    import concourse.bacc as bacc
    import concourse.mybir as mybir
    from concourse import tile

    F32 = mybir.dt.float32
    F32R = mybir.dt.float32r
    BF16 = mybir.dt.bfloat16
    ADD = mybir.AluOpType.add
    MUL = mybir.AluOpType.mult
    EXP = mybir.ActivationFunctionType.Exp

    nc = bacc.Bacc("TRN2", target_bir_lowering=False, debug=False, num_devices=NC_)

    QW = HL * HD                      # 256 local q / v feature cols
    qw_d = nc.dram_tensor("qw", [128, CT * QW], BF16, kind="ExternalInput")
    qt_d = nc.dram_tensor("qt", [128, CT * S], BF16, kind="ExternalInput")
    wvp_d = nc.dram_tensor("wvp", [128, CT * QW], BF16, kind="ExternalInput")
    vt_d = nc.dram_tensor("vt", [128, CT * S], BF16, kind="ExternalInput")
    kp_d = nc.dram_tensor("kp", [64, HL * S], F32R, kind="ExternalInput")
    g_d = nc.dram_tensor("g", [64, HL * S], F32R, kind="ExternalInput")
    wo_d = nc.dram_tensor("wo", [128, 2 * D], BF16, kind="ExternalInput")
    tabs_d = nc.dram_tensor("tabs", [128, 2 * S], BF16, kind="ExternalInput")
    vbc_d = nc.dram_tensor("vbc", [128, 4], F32, kind="ExternalInput")
    pm_d = nc.dram_tensor("pm", [128, 128], F32R, kind="ExternalInput")
    if has_bq:
        wq9_d = nc.dram_tensor("wq9", [1, QW], BF16, kind="ExternalInput")
    if has_bo:
        wo9_d = nc.dram_tensor("wo9", [1, D], BF16, kind="ExternalInput")
    out_d = nc.dram_tensor("out", [S, D], BF16, kind="ExternalOutput")

    with tile.TileContext(nc, num_cores=NC_) as tc:
      with tc.tile_pool(name="persist", bufs=1) as pp, \
           tc.tile_pool(name="scratch", bufs=2) as sp, \
           tc.tile_pool(name="exppool", bufs=34) as ep, \
           tc.tile_pool(name="normp", bufs=2) as np_, \
           tc.tile_pool(name="osb", bufs=4) as osb:

        # ---------- persistent SBUF ----------
        tabs = pp.tile([128, 2 * S], BF16, tag="tabs")
        vbc = pp.tile([128, 4], F32, tag="vbc")
        pm = pp.tile([128, 128], F32R, tag="pm")
        wqt = pp.tile([128, CT * QW], BF16, tag="wqt")
        qtt = pp.tile([128, CT * S], BF16, tag="qtt")
        kgt = pp.tile([128, HL * S], F32R, tag="kgt")
        wvall = pp.tile([128, CT * QW], BF16, tag="wvall")
        vtall = pp.tile([128, CT * S], BF16, tag="vtall")
        wot = pp.tile([128, 2 * D], BF16, tag="wot")
        catq = [pp.tile([128, S], F32R, name=f"catq{h}", tag=f"catq{h}")
                for h in range(HL)]
        vv = [pp.tile([128, HL * (HD + 1)], BF16, name=f"vv{j}", tag=f"vv{j}")
              for j in range(JT)]
        xn = [pp.tile([128, S], BF16, name=f"xn{c}", tag=f"xn{c}")
              for c in range(2)]
        ebias = pp.tile([128, 1], F32, tag="ebias")
        wsb = pp.tile([128, 512], F32R, tag="wsb")
        if has_bq:
            wq9 = pp.tile([1, QW], BF16, tag="wq9")
            oq = pp.tile([1, S], BF16, tag="oq")
        if has_bo:
            wo9 = pp.tile([1, D], BF16, tag="wo9")
            xn1 = pp.tile([1, 128], BF16, tag="xn1")

        # ---------- input DMAs: contiguous, big packets, 3 queues ----------
        # sync q1 (fast HW-DGE) carries the early-critical q path; the slow
        # gpsimd SW-DGE queue gets vt, which is needed last (vproj)
        # Each queue sustains a FIXED ~150B/ns independent of packet size
        # (transfers spray over all 16 DMA engines, ~10B/ns per queue per
        # engine), so bandwidth scales with ACTIVE QUEUES. Spread the
        # critical q-path across all three queues, each in need-order;
        # late tensors ride behind on the same queues (FIFO = free gating).
        # qt drips in per-c chunks split over both HW queues so qproj
        # starts as chunks land and the PE never idles into the HAM gate
        nc.sync.dma_start(wqt[:], qw_d.ap())
        for c in range(4):
            nc.sync.dma_start(qtt[:, c * S:(c + 1) * S],
                              qt_d.ap()[:, c * S:(c + 1) * S])
        nc.sync.dma_start(vtall[:, 0:4 * S], vt_d.ap()[:, 0:4 * S])
        nc.sync.dma_start(wvall[:], wvp_d.ap())
        if has_bq:
            nc.sync.dma_start(wq9[:], wq9_d.ap())

        nc.scalar.dma_start(tabs[:], tabs_d.ap())
        nc.scalar.dma_start(kgt[64:128, 0:S], g_d.ap()[:, 0:S])
        for c in range(4, 8):
            nc.scalar.dma_start(qtt[:, c * S:(c + 1) * S],
                                qt_d.ap()[:, c * S:(c + 1) * S])
        nc.scalar.dma_start(vtall[:, 4 * S:8 * S], vt_d.ap()[:, 4 * S:8 * S])
        nc.scalar.dma_start(wot[:], wo_d.ap())
        if has_bo:
            nc.scalar.dma_start(wo9[:], wo9_d.ap())

        nc.gpsimd.dma_start(kgt[0:64, :], kp_d.ap())
        for h in range(1, HL):
            nc.gpsimd.dma_start(kgt[64:128, h * S:(h + 1) * S],
                                g_d.ap()[:, h * S:(h + 1) * S])
        nc.gpsimd.dma_start(vbc[:], vbc_d.ap())
        nc.gpsimd.dma_start(pm[:], pm_d.ap())

        # views
        kg = [kgt[:, h * S:(h + 1) * S] for h in range(HL)]
        wv = [wvall[:, c * QW:(c + 1) * QW] for c in range(CT)]
        vt = [vtall[:, c * S:(c + 1) * S] for c in range(CT)]
        wo = [wot[:, c * D:(c + 1) * D] for c in range(2)]
        coss = tabs[:, 0:S]
        sinsw = tabs[:, S:2 * S]

        # ---------- small inits ----------
        nc.vector.memset(ebias[:], -25.0)
        nc.vector.memset(wsb[:].bitcast(F32), 0.01)
        one_pair = float(np.array([0x3F803F80], np.uint32).view(np.float32)[0])
        if has_bq:
            nc.vector.memset(oq[:].bitcast(F32), one_pair)
        if has_bo:
            nc.vector.memset(xn1[:].bitcast(F32), one_pair)
        # fill vv with bf16 1.0s (the float below is two bf16 1.0s); the
        # vproj evictions overwrite the v columns, leaving the ones columns
        for j in range(JT):
            nc.vector.memset(vv[j][:].bitcast(F32), one_pair)

        # PSUM pools open/close in LIFO phase order within the 8-bank
        # budget: [sc 4 + qp 4] -> vp 2 -> xt 4.
        scps = tc.alloc_tile_pool(name="scps", bufs=2, space="PSUM")
        qps = tc.alloc_tile_pool(name="qps", bufs=1, space="PSUM")

        # ---------- q projection + rotation, per column half ----------
        def qproj_half(ft, qp, half):
            hs = slice(half * 512, half * 512 + 512)
            for c in range(CT):
                nc.tensor.matmul(
                    qp[:, hs],
                    wqt[:, c * QW + ft * 128:c * QW + (ft + 1) * 128],
                    qtt[:, c * S + half * 512:c * S + half * 512 + 512],
                    start=(c == 0), stop=(c == CT - 1 and not has_bq),
                    skip_group_check=True)
            if has_bq:
                nc.tensor.matmul(
                    qp[:, hs], wq9[:, ft * 128:(ft + 1) * 128],
                    oq[:, hs], start=False, stop=True, skip_group_check=True)

        def rot_half(ft, qp, qp2, half):
            hs = slice(half * 512, half * 512 + 512)
            # swapped operand: evict qp to SBUF, permute on the PE
            qsb = sp.tile([128, 512], F32R, tag="qsb")
            nc.scalar.copy(qsb[:], qp[:, hs])
            nc.tensor.matmul(qp2[:, hs], pm[:], qsb[:],
                             start=True, stop=True, skip_group_check=True)
            # q rows -> catq[0:64] (DVE partition-shift copies; keeps ACT
            # free for the exp stream)
            nc.vector.tensor_copy(catq[2 * ft][0:64, hs], qp[0:64, hs])
            nc.vector.tensor_copy(catq[2 * ft + 1][0:64, hs], qp[64:128, hs])
            # rotation -> catq rows 64:128:
            #   t1 = (qp + vb) * cos(w s);  u = (qp2 + vb_sw) * sinsw
            #   catq[64:128] = t1 + u   (sin sign folded into sinsw)
            t1 = sp.tile([128, 512], F32, tag="t1")
            nc.vector.scalar_tensor_tensor(
                out=t1[:], in0=qp[:, hs],
                scalar=vbc[:, ft:ft + 1],
                in1=coss[:, hs], op0=ADD, op1=MUL)
            u_ = sp.tile([128, 512], F32, tag="u_")
            nc.vector.scalar_tensor_tensor(
                out=u_[:], in0=qp2[:, hs],
                scalar=vbc[:, 2 + ft:2 + ft + 1],
                in1=sinsw[:, hs], op0=ADD, op1=MUL)
            for par in range(2):
                hq = 2 * ft + par
                o_ = par * 64
                nc.vector.tensor_tensor(
                    out=catq[hq][64:128, hs], in0=t1[o_:o_ + 64, :],
                    in1=u_[o_:o_ + 64, :], op=ADD)

        def qproj_rot(ft, qp, qp2):
            for half in range(2):
                qproj_half(ft, qp, half)
                rot_half(ft, qp, qp2, half)

        def scores_exp(h, jt_):
            sc = scps.tile([128, S], F32, tag="sc")
            for half in range(2):
                hs = slice(half * 512, half * 512 + 512)
                nc.tensor.matmul(
                    sc[:, hs], kg[h][:, jt_ * 128:(jt_ + 1) * 128],
                    catq[h][:, hs], start=True, stop=True,
                    skip_group_check=True)
            ex = ep.tile([128, S], BF16, tag="ex")
            nc.scalar.activation(ex[:], sc[:], EXP, bias=ebias[:], scale=1.0)
            return ex

        def attnv(h, jt_, ex, xt):
            for half in range(2):
                hs = slice(half * 512, half * 512 + 512)
                nc.tensor.matmul(
                    xt[0:HD + 1, hs],
                    vv[jt_][:, h * (HD + 1):(h + 1) * (HD + 1)], ex[:, hs],
                    start=(jt_ == 0), stop=(jt_ == JT - 1),
                    skip_group_check=True)

        # normalize is software-pipelined in two stages so the DVE queue
        # never has head h's multiply blocking head h+1's reciprocal:
        # stage A (copy denom row, reciprocal, broadcast) runs right after
        # each head's attnv; stage B (the multiply) is deferred one head.
        def norm_a(h, xt):
            dsb = np_.tile([1, S], F32, tag="dsb")
            rsb = np_.tile([1, S], F32, tag="rsb")
            rbs = np_.tile([64, S], F32, tag="rbs")
            for half in range(2):
                hs = slice(half * 512, half * 512 + 512)
                nc.scalar.copy(dsb[0:1, hs], xt[HD:HD + 1, hs])
                nc.vector.reciprocal_approx_fast(out=rsb[0:1, hs],
                                                 in_=dsb[0:1, hs])
                nc.gpsimd.partition_broadcast(rbs[:, hs], rsb[0:1, hs])
            return rbs

        def norm_b(h, xt, rbs):
            nc.vector.tensor_tensor(
                out=xn[h // 2][(h % 2) * 64:(h % 2) * 64 + 64, :],
                in0=xt[0:HD, :], in1=rbs[:, :], op=MUL)

        # ---------- emission ----------
        # pre-warm: full-array dummies into the qp buffer before its first
        # real use, so the HAM clock gate is released before qproj starts
        wtile = qps.tile([128, S], F32, tag="qp")
        for _ in range(14):
            nc.tensor.matmul(wtile[0:128, 0:512], wsb[:, 0:128], wsb[:, :],
                             start=True, stop=True, skip_group_check=True)
        qp0 = qps.tile([128, S], F32, tag="qp")
        qp20 = qps.tile([128, S], F32, tag="qp2")
        qproj_rot(0, qp0, qp20)
        exh0 = [scores_exp(0, jt_) for jt_ in range(JT)]

        qp1 = qps.tile([128, S], F32, tag="qp")
        qp21 = qps.tile([128, S], F32, tag="qp2")
        qproj_rot(1, qp1, qp21)
        qps.release()

        # scores/exp for heads 1-3 interleave with vproj and attnv h0/h1:
        # the exp stream (ACT, ~1.1us per tile) is slower than the 2 score
        # matmuls per tile, so the PE fills the difference with real work
        # instead of stalling on the sc-pool
        vps = tc.alloc_tile_pool(name="vps", bufs=2, space="PSUM")
        exh1, exh2, exh3 = [], [], []
        for jt_ in range(JT):
            exh1.append(scores_exp(1, jt_))
            vp = vps.tile([128, QW], F32, tag="vp")
            for c in range(CT):
                nc.tensor.matmul(
                    vp[:], vt[c][:, jt_ * 128:(jt_ + 1) * 128], wv[c][:],
                    start=(c == 0), stop=(c == CT - 1),
                    skip_group_check=True)
            dst = vv[jt_][:].rearrange("p (h x) -> p h x", x=HD + 1)[:, :, 0:HD]
            src_ = vp[:].rearrange("p (h d) -> p h d", d=HD)
            nc.vector.tensor_copy(dst, src_)
        vps.release()

        xtps = tc.alloc_tile_pool(name="xtps", bufs=2, space="PSUM")
        xts, rbss = [], []

        xt0 = xtps.tile([128, S], F32, tag="xt")
        xts.append(xt0)
        for jt_ in range(JT):
            exh2.append(scores_exp(2, jt_))
            attnv(0, jt_, exh0[jt_], xt0)
        rbss.append(norm_a(0, xt0))

        xt1 = xtps.tile([128, S], F32, tag="xt")
        xts.append(xt1)
        for jt_ in range(JT):
            exh3.append(scores_exp(3, jt_))
            attnv(1, jt_, exh1[jt_], xt1)
        rbss.append(norm_a(1, xt1))
        norm_b(0, xts[0], rbss[0])

        for h in (2, 3):
            xt = xtps.tile([128, S], F32, tag="xt")
            xts.append(xt)
            exh = (exh2, exh3)[h - 2]
            for jt_ in range(JT):
                attnv(h, jt_, exh[jt_], xt)
            rbss.append(norm_a(h, xt))
            norm_b(h - 1, xts[h - 1], rbss[h - 1])
        norm_b(HL - 1, xts[HL - 1], rbss[HL - 1])

        # bridge the last normalize's latency so the PE stays at full clock
        # into the output projection (tile reuses an early xt slot whose
        # reads are long done)
        dtile = xtps.tile([128, S], F32, tag="xt")
        for _ in range(8):
            nc.tensor.matmul(dtile[0:128, 0:512], wsb[:, 0:128], wsb[:, :],
                             start=True, stop=True, skip_group_check=True)
        xtps.release()
        scps.release()

        # ---------- output projection (partial out, bf16) ----------
        ops = tc.alloc_tile_pool(name="ops", bufs=3, space="PSUM")
        dengs = [nc.sync, nc.gpsimd, nc.scalar]
        for qt in range(8):
            op = ops.tile([128, D], F32, tag="op")
            for half in range(2):
                hs = slice(half * 512, half * 512 + 512)
                for c in range(2):
                    nc.tensor.matmul(
                        op[:, hs], xn[c][:, qt * 128:(qt + 1) * 128],
                        wo[c][:, hs], start=(c == 0),
                        stop=(c == 1 and not has_bo), skip_group_check=True)
                if has_bo:
                    nc.tensor.matmul(
                        op[:, hs], xn1[:], wo9[:, hs],
                        start=False, stop=True, skip_group_check=True)
            os_ = osb.tile([128, D], BF16, tag="os")
            # split evictions across ACT + DVE so the PSUM bank frees at
            # ~2x single-engine copy rate
            nc.scalar.copy(os_[:, 0:512], op[:, 0:512])
            nc.vector.tensor_copy(os_[:, 512:1024], op[:, 512:1024])
            dengs[qt % 3].dma_start(out_d.ap()[qt * 128:(qt + 1) * 128, :],
                                    os_[:])
        ops.release()

    nc.finalize()
    return nc


def _host_pack(query, key, value, Wq, bq, Wv, bv, Wo, bo, v_bias):
    """Build the 8 per-core input maps (tensors pre-packed in SBUF layout)."""
    import ml_dtypes
    r = _rne_fp32r
    bf = ml_dtypes.bfloat16
    QW = HL * HD
    w = np.exp(np.arange(HALF) * (-math.log(10000.0) / (HALF - 1)))

    has_bq = bool(np.any(bq))
    has_bo = bool(np.any(bo)) or bool(np.any(bv))

    # tables shared across the 4 cores of a batch except vb cols
    j = np.arange(S, dtype=np.float64)
    ang_j = w[:, None] * j[None, :]                      # [32, S]
    g64 = np.concatenate([np.sin(ang_j), np.cos(ang_j)], axis=0).astype(np.float32)
    g4 = r(np.tile(g64, (1, HL)))                        # [64, HL*S]

    wrep = np.tile(w, 4)[:, None]                        # [128, 1]
    svals = np.arange(S, dtype=np.float64)[None, :]
    cos_ws = np.cos(wrep * svals).astype(np.float32)     # [128, S]
    sin_ws = np.sin(wrep * svals).astype(np.float32)
    # u[p] = (qp2 + vb_sw)[p] * sinsw[p] must give +sin for rows p%64<32
    # (a_sin) and -sin for rows p%64>=32 (a_cos)
    sinsw = sin_ws.copy()
    sinsw[32:64] *= -1.0
    sinsw[96:128] *= -1.0

    # within-head swap of the 32-dim halves (for the rotation's sin term)
    sw_idx = np.arange(HL * HD)
    sw_idx = (sw_idx // HD) * HD + ((sw_idx % HD) + HALF) % HD
    sw128 = np.arange(128)
    sw128 = (sw128 // HD) * HD + ((sw128 % HD) + HALF) % HD
    pmat = np.zeros((128, 128), np.float32)
    pmat[sw128, np.arange(128)] = 1.0                    # P[p,i]=1 iff p=sw(i)

    WqT = Wq.T.astype(np.float32)                        # [D, D]
    WvT = Wv.T.astype(np.float32)
    WoT = Wo.T.astype(np.float32)                        # [Dv, D]

    qTs, vTs, kTs = [], [], []
    for b in range(B):
        qTs.append(_pack_cpn(query[b].T).astype(bf))     # [128, CT*S]
        vTs.append(_pack_cpn(value[b].T).astype(bf))
        kTs.append(np.ascontiguousarray(key[b].T))

    in_maps = []
    for c in range(NC_):
        b, g = c // 4, c % 4
        col0 = g * QW

        bql = bq[col0:col0 + QW]
        vbl = v_bias.reshape(-1)[col0:col0 + QW].astype(np.float32)
        WqTl = WqT[:, col0:col0 + QW]

        kp = kTs[b][col0:col0 + QW]                      # [256, S]
        kpp = np.ascontiguousarray(
            kp.reshape(HL, 64, S).transpose(1, 0, 2).reshape(64, HL * S))

        wop = _pack_cpn(WoT[col0:col0 + QW, :])          # [128, 2*D]

        vbl_sw = vbl[sw_idx]
        tabs = np.empty((128, 2 * S), np.float32)
        tabs[:, 0:S] = cos_ws
        tabs[:, S:2 * S] = sinsw
        vbcols = np.empty((128, 4), np.float32)
        for ft in range(2):
            vbcols[:, ft] = vbl[ft * 128:(ft + 1) * 128]
            vbcols[:, 2 + ft] = vbl_sw[ft * 128:(ft + 1) * 128]

        im = {
            "qw": _pack_cpn(WqTl).astype(bf),
            "qt": qTs[b],
            "wvp": _pack_cpn(WvT[:, col0:col0 + QW]).astype(bf),
            "vt": vTs[b],
            "kp": r(kpp),
            "g": g4,
            "wo": wop.astype(bf),
            "tabs": tabs.astype(bf),
            "vbc": vbcols,
            "pm": pmat,
        }
        if has_bq:
            im["wq9"] = bql[None, :].astype(bf)
        if has_bo:
            im["wo9"] = (bo / 4.0
                         + bv[col0:col0 + QW] @ WoT[col0:col0 + QW, :]
                         )[None, :].astype(bf)
        in_maps.append(im)
    return in_maps, has_bq, has_bo


def kernel(query, key, value, mask, Wq, bq, Wv, bv, Wo, bo, v_bias):
    from concourse.bass_utils import run_bass_kernel_spmd

    query = np.asarray(query, np.float32)
    key = np.asarray(key, np.float32)
    value = np.asarray(value, np.float32)
    in_maps, has_bq, has_bo = _host_pack(
        query, key, value,
        np.asarray(Wq, np.float32), np.asarray(bq, np.float32),
        np.asarray(Wv, np.float32), np.asarray(bv, np.float32),
        np.asarray(Wo, np.float32), np.asarray(bo, np.float32),
        np.asarray(v_bias, np.float32))

    ckey = ("nc", has_bq, has_bo)
    if ckey not in _cache:
        _cache[ckey] = _build_nc(has_bq, has_bo)
    nc = _cache[ckey]

    res = run_bass_kernel_spmd(
        nc, in_maps, core_ids=list(range(NC_)),
        trace=bool(int(os.environ.get("BASS_KERNEL_TRACE", "0"))))
    _cache["last_result"] = res

    out = np.empty((B, S, D), np.float32)
    for b in range(B):
        acc = res.results[4 * b]["out"].astype(np.float32)
        for g in range(1, 4):
            acc = acc + res.results[4 * b + g]["out"].astype(np.float32)
        out[b] = acc
    return out


# revision 47
# speedup vs baseline: 1.0826x; 1.0826x over previous
"""TENER-style MultiHeadedAttention TRN2 kernel (8 NeuronCores, SPMD).

Sharding (tensor-parallel over heads x data-parallel over batch):
core c handles batch b = c//4 and the 4 heads [4*(c%4), 4*(c%4)+4),
over ALL 1024 query rows. Wq/Wv are split column-wise by head, Wo
row-wise; each core emits a PARTIAL output [S, D] and the host gather
sums the 4 partials per batch (the Wo all-reduce).

Key math: the TENER relative-position term after the shift trick is
  rel[s, j] = (q_s + v_bias_h) . pos[j - s]
and pos rows are sinusoids, so by angle addition the whole score is ONE
128-deep contraction per head:
  scores[j, s] = [k_j ; sin(w j) ; cos(w j)] . [q_s ; a_sin(s) ; a_cos(s)]
  a_sin = qv_sin*cos(w s) + qv_cos*sin(w s)
  a_cos = qv_cos*cos(w s) - qv_sin*sin(w s)

The swapped-q operand of the rotation (qv_cos paired with sin rows and
vice versa) is built on-chip: evict qp PSUM->SBUF (ACT), multiply by a
128x128 permutation matrix on the PE (one matmul per ft/half) instead
of a second full projection chain.

All inputs are HOST-PACKED into their final SBUF layouts so every load
is one contiguous DMA with multi-KB packets: the dynamic DMA queues
share one engine round-robin by packet, so small-element rearranging
DMAs starve behind large ones. Loads spread over three queues (sync q1
+ scalar q10 HW-DGE, gpsimd q0 SW-DGE).

All q/k-path matmuls run as float32r (full PE rate at free-dim >= 256).
The v projection runs in bf16. Softmax denominators come free via a
ones-column per head appended to v; normalization runs one
reciprocal (DVE, straight off PSUM) + partition-broadcast (gpsimd) +
multiply (DVE) per head, overlapped under the next head's attnv.
"""

import math
import os
import sys

sys.path.insert(0, "/opt/trn_rl_repo")

import numpy as np

B, S, D = 2, 1024, 1024
H, HD = 16, 64          # global heads, head_dim
HL = 4                  # local heads per core
HALF = 32               # sin/cos half of head_dim
NC_ = 8                 # cores
JT = S // 128            # 8 key tiles
CT = D // 128            # 8 contraction tiles
TB = 2 * S + 4           # tabs: coss | sinsw | vb cols(2) | vb_sw cols(2)

_cache: dict = {}


def _rne_fp32r(a):
    """Round fp32 -> fp32r (1s+8e+11m) with round-to-nearest-even."""
    u = np.ascontiguousarray(a, dtype=np.float32).view(np.uint32)
    lsb = (u >> np.uint32(12)) & np.uint32(1)
    return ((u + np.uint32(0x7FF) + lsb) & np.uint32(0xFFFFF000)).view(np.float32)


def _pack_cpn(m, p=128):
    """[C*p, N] -> [p, C*N] (SBUF layout: partition-major, c-blocks along free)."""
    cp, n = m.shape
    c = cp // p
    return np.ascontiguousarray(
        m.reshape(c, p, n).transpose(1, 0, 2).reshape(p, c * n))


def _build_nc(has_bq: bool, has_bo: bool):
    import concourse.bacc as bacc
    import concourse.mybir as mybir
    from concourse import tile

    F32 = mybir.dt.float32
    F32R = mybir.dt.float32r
    BF16 = mybir.dt.bfloat16
    ADD = mybir.AluOpType.add
    MUL = mybir.AluOpType.mult
    EXP = mybir.ActivationFunctionType.Exp

    nc = bacc.Bacc("TRN2", target_bir_lowering=False, debug=False, num_devices=NC_)

    QW = HL * HD                      # 256 local q / v feature cols
    qw_d = nc.dram_tensor("qw", [128, CT * QW], BF16, kind="ExternalInput")
    qt_d = nc.dram_tensor("qt", [128, CT * S], BF16, kind="ExternalInput")
    wvp_d = nc.dram_tensor("wvp", [128, CT * QW], BF16, kind="ExternalInput")
    vt_d = nc.dram_tensor("vt", [128, CT * S], BF16, kind="ExternalInput")
    kp_d = nc.dram_tensor("kp", [64, HL * S], F32R, kind="ExternalInput")
    g_d = nc.dram_tensor("g", [64, HL * S], F32R, kind="ExternalInput")
    wo_d = nc.dram_tensor("wo", [128, 2 * D], BF16, kind="ExternalInput")
    tabs_d = nc.dram_tensor("tabs", [128, 2 * S], BF16, kind="ExternalInput")
    vbc_d = nc.dram_tensor("vbc", [128, 4], F32, kind="ExternalInput")
    pm_d = nc.dram_tensor("pm", [128, 128], F32R, kind="ExternalInput")
    if has_bq:
        wq9_d = nc.dram_tensor("wq9", [1, QW], BF16, kind="ExternalInput")
    if has_bo:
        wo9_d = nc.dram_tensor("wo9", [1, D], BF16, kind="ExternalInput")
    out_d = nc.dram_tensor("out", [S, D], BF16, kind="ExternalOutput")

    with tile.TileContext(nc, num_cores=NC_) as tc:
      with tc.tile_pool(name="persist", bufs=1) as pp, \
           tc.tile_pool(name="scratch", bufs=2) as sp, \
           tc.tile_pool(name="exppool", bufs=34) as ep, \
           tc.tile_pool(name="normp", bufs=2) as np_, \
           tc.tile_pool(name="osb", bufs=4) as osb:

        # ---------- persistent SBUF ----------
        tabs = pp.tile([128, 2 * S], BF16, tag="tabs")
        vbc = pp.tile([128, 4], F32, tag="vbc")
        pm = pp.tile([128, 128], F32R, tag="pm")
        wqt = pp.tile([128, CT * QW], BF16, tag="wqt")
        qtt = pp.tile([128, CT * S], BF16, tag="qtt")
        kgt = pp.tile([128, HL * S], F32R, tag="kgt")
        wvall = pp.tile([128, CT * QW], BF16, tag="wvall")
        vtall = pp.tile([128, CT * S], BF16, tag="vtall")
        wot = pp.tile([128, 2 * D], BF16, tag="wot")
        catq = [pp.tile([128, S], F32R, name=f"catq{h}", tag=f"catq{h}")
                for h in range(HL)]
        vv = [pp.tile([128, HL * (HD + 1)], BF16, name=f"vv{j}", tag=f"vv{j}")
              for j in range(JT)]
        xn = [pp.tile([128, S], BF16, name=f"xn{c}", tag=f"xn{c}")
              for c in range(2)]
        ebias = pp.tile([128, 1], F32, tag="ebias")
        wsb = pp.tile([128, 512], F32R, tag="wsb")
        if has_bq:
            wq9 = pp.tile([1, QW], BF16, tag="wq9")
            oq = pp.tile([1, S], BF16, tag="oq")
        if has_bo:
            wo9 = pp.tile([1, D], BF16, tag="wo9")
            xn1 = pp.tile([1, 128], BF16, tag="xn1")

        # ---------- input DMAs: contiguous, big packets, 3 queues ----------
        # sync q1 (fast HW-DGE) carries the early-critical q path; the slow
        # gpsimd SW-DGE queue gets vt, which is needed last (vproj)
        # Each queue sustains a FIXED ~150B/ns independent of packet size
        # (transfers spray over all 16 DMA engines, ~10B/ns per queue per
        # engine), so bandwidth scales with ACTIVE QUEUES. Spread the
        # critical q-path across all three queues, each in need-order;
        # late tensors ride behind on the same queues (FIFO = free gating).
        # Per-queue DMA rate ~ element_size/(20ns + size/167B/ns): big
        # per-partition elements matter, queues run concurrently. Split qt
        # (the first PE dependency) over both HW queues as 8KB-element
        # halves; kp/g chunks ride the slow SW queue in scores-head order.
        nc.sync.dma_start(wqt[:], qw_d.ap())
        nc.sync.dma_start(qtt[:, 0:4 * S], qt_d.ap()[:, 0:4 * S])
        nc.sync.dma_start(vtall[:, 0:4 * S], vt_d.ap()[:, 0:4 * S])
        nc.sync.dma_start(wvall[:], wvp_d.ap())
        if has_bq:
            nc.sync.dma_start(wq9[:], wq9_d.ap())

        nc.scalar.dma_start(tabs[:], tabs_d.ap())
        nc.scalar.dma_start(qtt[:, 4 * S:8 * S], qt_d.ap()[:, 4 * S:8 * S])
        nc.scalar.dma_start(vtall[:, 4 * S:8 * S], vt_d.ap()[:, 4 * S:8 * S])
        nc.scalar.dma_start(wot[:], wo_d.ap())
        if has_bo:
            nc.scalar.dma_start(wo9[:], wo9_d.ap())

        nc.gpsimd.dma_start(kgt[0:64, :], kp_d.ap())
        for h in range(HL):
            nc.gpsimd.dma_start(kgt[64:128, h * S:(h + 1) * S],
                                g_d.ap()[:, h * S:(h + 1) * S])
        nc.gpsimd.dma_start(vbc[:], vbc_d.ap())
        nc.gpsimd.dma_start(pm[:], pm_d.ap())

        # views
        kg = [kgt[:, h * S:(h + 1) * S] for h in range(HL)]
        wv = [wvall[:, c * QW:(c + 1) * QW] for c in range(CT)]
        vt = [vtall[:, c * S:(c + 1) * S] for c in range(CT)]
        wo = [wot[:, c * D:(c + 1) * D] for c in range(2)]
        coss = tabs[:, 0:S]
        sinsw = tabs[:, S:2 * S]

        # ---------- small inits ----------
        nc.vector.memset(ebias[:], -25.0)
        nc.vector.memset(wsb[:].bitcast(F32), 0.01)
        one_pair = float(np.array([0x3F803F80], np.uint32).view(np.float32)[0])
        if has_bq:
            nc.vector.memset(oq[:].bitcast(F32), one_pair)
        if has_bo:
            nc.vector.memset(xn1[:].bitcast(F32), one_pair)
        # fill vv with bf16 1.0s (the float below is two bf16 1.0s); the
        # vproj evictions overwrite the v columns, leaving the ones columns
        for j in range(JT):
            nc.vector.memset(vv[j][:].bitcast(F32), one_pair)

        # PSUM pools open/close in LIFO phase order within the 8-bank
        # budget: [sc 4 + qp 4] -> vp 2 -> xt 4.
        scps = tc.alloc_tile_pool(name="scps", bufs=2, space="PSUM")
        qps = tc.alloc_tile_pool(name="qps", bufs=1, space="PSUM")

        # ---------- q projection + rotation, per column half ----------
        def qproj_half(ft, qp, half):
            hs = slice(half * 512, half * 512 + 512)
            for c in range(CT):
                nc.tensor.matmul(
                    qp[:, hs],
                    wqt[:, c * QW + ft * 128:c * QW + (ft + 1) * 128],
                    qtt[:, c * S + half * 512:c * S + half * 512 + 512],
                    start=(c == 0), stop=(c == CT - 1 and not has_bq),
                    skip_group_check=True)
            if has_bq:
                nc.tensor.matmul(
                    qp[:, hs], wq9[:, ft * 128:(ft + 1) * 128],
                    oq[:, hs], start=False, stop=True, skip_group_check=True)

        def rot_half(ft, qp, qp2, half):
            hs = slice(half * 512, half * 512 + 512)
            # swapped operand: evict qp to SBUF, permute on the PE
            qsb = sp.tile([128, 512], F32R, tag="qsb")
            nc.scalar.copy(qsb[:], qp[:, hs])
            nc.tensor.matmul(qp2[:, hs], pm[:], qsb[:],
                             start=True, stop=True, skip_group_check=True)
            # q rows -> catq[0:64] (DVE partition-shift copies; keeps ACT
            # free for the exp stream)
            nc.vector.tensor_copy(catq[2 * ft][0:64, hs], qp[0:64, hs])
            nc.vector.tensor_copy(catq[2 * ft + 1][0:64, hs], qp[64:128, hs])
            # rotation -> catq rows 64:128:
            #   t1 = (qp + vb) * cos(w s);  u = (qp2 + vb_sw) * sinsw
            #   catq[64:128] = t1 + u   (sin sign folded into sinsw)
            t1 = sp.tile([128, 512], F32, tag="t1")
            nc.vector.scalar_tensor_tensor(
                out=t1[:], in0=qp[:, hs],
                scalar=vbc[:, ft:ft + 1],
                in1=coss[:, hs], op0=ADD, op1=MUL)
            u_ = sp.tile([128, 512], F32, tag="u_")
            nc.vector.scalar_tensor_tensor(
                out=u_[:], in0=qp2[:, hs],
                scalar=vbc[:, 2 + ft:2 + ft + 1],
                in1=sinsw[:, hs], op0=ADD, op1=MUL)
            for par in range(2):
                hq = 2 * ft + par
                o_ = par * 64
                nc.vector.tensor_tensor(
                    out=catq[hq][64:128, hs], in0=t1[o_:o_ + 64, :],
                    in1=u_[o_:o_ + 64, :], op=ADD)

        def qproj_rot(ft, qp, qp2):
            for half in range(2):
                qproj_half(ft, qp, half)
                rot_half(ft, qp, qp2, half)

        def scores_exp(h, jt_, pool=None):
            sc = (pool or scps).tile([128, S], F32, tag="sc")
            for half in range(2):
                hs = slice(half * 512, half * 512 + 512)
                nc.tensor.matmul(
                    sc[:, hs], kg[h][:, jt_ * 128:(jt_ + 1) * 128],
                    catq[h][:, hs], start=True, stop=True,
                    skip_group_check=True)
            ex = ep.tile([128, S], BF16, tag="ex")
            nc.scalar.activation(ex[:], sc[:], EXP, bias=ebias[:], scale=1.0)
            return ex

        def attnv(h, jt_, ex, xt):
            for half in range(2):
                hs = slice(half * 512, half * 512 + 512)
                nc.tensor.matmul(
                    xt[0:HD + 1, hs],
                    vv[jt_][:, h * (HD + 1):(h + 1) * (HD + 1)], ex[:, hs],
                    start=(jt_ == 0), stop=(jt_ == JT - 1),
                    skip_group_check=True)

        # normalize is software-pipelined in two stages so the DVE queue
        # never has head h's multiply blocking head h+1's reciprocal:
        # stage A (copy denom row, reciprocal, broadcast) runs right after
        # each head's attnv; stage B (the multiply) is deferred one head.
        def norm_a(h, xt):
            dsb = np_.tile([1, S], F32, tag="dsb")
            rsb = np_.tile([1, S], F32, tag="rsb")
            rbs = np_.tile([64, S], F32, tag="rbs")
            for half in range(2):
                hs = slice(half * 512, half * 512 + 512)
                nc.vector.tensor_copy(dsb[0:1, hs], xt[HD:HD + 1, hs])
                nc.vector.reciprocal_approx_fast(out=rsb[0:1, hs],
                                                 in_=dsb[0:1, hs])
                nc.gpsimd.partition_broadcast(rbs[:, hs], rsb[0:1, hs])
            return rbs

        def norm_b(h, xt, rbs):
            nc.vector.tensor_tensor(
                out=xn[h // 2][(h % 2) * 64:(h % 2) * 64 + 64, :],
                in0=xt[0:HD, :], in1=rbs[:, :], op=MUL)

        # ---------- emission ----------
        # pre-warm: full-array dummies into the qp buffer before its first
        # real use, so the HAM clock gate is released before qproj starts
        wtile = qps.tile([128, S], F32, tag="qp")
        for _ in range(28):
            nc.tensor.matmul(wtile[0:128, 0:512], wsb[:, 0:128], wsb[:, :],
                             start=True, stop=True, skip_group_check=True)
        qp0 = qps.tile([128, S], F32, tag="qp")
        qp20 = qps.tile([128, S], F32, tag="qp2")
        qproj_rot(0, qp0, qp20)
        exh0 = [scores_exp(0, jt_) for jt_ in range(JT)]

        qp1 = qps.tile([128, S], F32, tag="qp")
        qp21 = qps.tile([128, S], F32, tag="qp2")
        qproj_rot(1, qp1, qp21)
        qps.release()

        exh1 = [scores_exp(1, jt_) for jt_ in range(JT)]
        exh2 = [scores_exp(2, jt_) for jt_ in range(JT)]
        exh3 = [scores_exp(3, jt_) for jt_ in range(JT)]

        # v projection (pure PE work) fills the window while the ACT exp
        # stream catches up on heads 0-3
        vps = tc.alloc_tile_pool(name="vps", bufs=2, space="PSUM")
        for jt_ in range(JT):
            vp = vps.tile([128, QW], F32, tag="vp")
            for c in range(CT):
                nc.tensor.matmul(
                    vp[:], vt[c][:, jt_ * 128:(jt_ + 1) * 128], wv[c][:],
                    start=(c == 0), stop=(c == CT - 1),
                    skip_group_check=True)
            dst = vv[jt_][:].rearrange("p (h x) -> p h x", x=HD + 1)[:, :, 0:HD]
            src_ = vp[:].rearrange("p (h d) -> p h d", d=HD)
            nc.vector.tensor_copy(dst, src_)
        vps.release()
        scps.release()

        # attnv pairs interleaved per key tile; all four xt accumulators
        # live at once so normalize never gates attnv via PSUM slot reuse
        xtps = tc.alloc_tile_pool(name="xtps", bufs=4, space="PSUM")
        xts, rbss = [], []
        xt0 = xtps.tile([128, S], F32, tag="xt")
        xt1 = xtps.tile([128, S], F32, tag="xt")
        for jt_ in range(JT):
            attnv(0, jt_, exh0[jt_], xt0)
            attnv(1, jt_, exh1[jt_], xt1)
        rbss.append(norm_a(0, xt0))
        rbss.append(norm_a(1, xt1))
        norm_b(0, xt0, rbss[0])
        norm_b(1, xt1, rbss[1])

        xt2 = xtps.tile([128, S], F32, tag="xt")
        xt3 = xtps.tile([128, S], F32, tag="xt")
        for jt_ in range(JT):
            attnv(2, jt_, exh2[jt_], xt2)
            attnv(3, jt_, exh3[jt_], xt3)
        rbss.append(norm_a(2, xt2))
        rbss.append(norm_a(3, xt3))
        norm_b(2, xt2, rbss[2])
        norm_b(3, xt3, rbss[3])

        # bridge the last normalize's latency so the PE stays at full clock
        # into the output projection (tile reuses xt0's slot; its reads are
        # long done)
        dtile = xtps.tile([128, S], F32, tag="xt")
        for _ in range(8):
            nc.tensor.matmul(dtile[0:128, 0:512], wsb[:, 0:128], wsb[:, :],
                             start=True, stop=True, skip_group_check=True)
        xtps.release()

        # ---------- output projection (partial out, bf16) ----------
        ops = tc.alloc_tile_pool(name="ops", bufs=3, space="PSUM")
        dengs = [nc.sync, nc.gpsimd, nc.scalar]
        for qt in range(8):
            op = ops.tile([128, D], F32, tag="op")
            for half in range(2):
                hs = slice(half * 512, half * 512 + 512)
                for c in range(2):
                    nc.tensor.matmul(
                        op[:, hs], xn[c][:, qt * 128:(qt + 1) * 128],
                        wo[c][:, hs], start=(c == 0),
                        stop=(c == 1 and not has_bo), skip_group_check=True)
                if has_bo:
                    nc.tensor.matmul(
                        op[:, hs], xn1[:], wo9[:, hs],
                        start=False, stop=True, skip_group_check=True)
            os_ = osb.tile([128, D], BF16, tag="os")
            # split evictions across ACT + DVE so the PSUM bank frees at
            # ~2x single-engine copy rate
            nc.scalar.copy(os_[:, 0:512], op[:, 0:512])
            nc.vector.tensor_copy(os_[:, 512:1024], op[:, 512:1024])
            dengs[qt % 3].dma_start(out_d.ap()[qt * 128:(qt + 1) * 128, :],
                                    os_[:])
        ops.release()

    nc.finalize()
    return nc


def _host_pack(query, key, value, Wq, bq, Wv, bv, Wo, bo, v_bias):
    """Build the 8 per-core input maps (tensors pre-packed in SBUF layout)."""
    import ml_dtypes
    r = _rne_fp32r
    bf = ml_dtypes.bfloat16
    QW = HL * HD
    w = np.exp(np.arange(HALF) * (-math.log(10000.0) / (HALF - 1)))

    has_bq = bool(np.any(bq))
    has_bo = bool(np.any(bo)) or bool(np.any(bv))

    # tables shared across the 4 cores of a batch except vb cols
    j = np.arange(S, dtype=np.float64)
    ang_j = w[:, None] * j[None, :]                      # [32, S]
    g64 = np.concatenate([np.sin(ang_j), np.cos(ang_j)], axis=0).astype(np.float32)
    g4 = r(np.tile(g64, (1, HL)))                        # [64, HL*S]

    wrep = np.tile(w, 4)[:, None]                        # [128, 1]
    svals = np.arange(S, dtype=np.float64)[None, :]
    cos_ws = np.cos(wrep * svals).astype(np.float32)     # [128, S]
    sin_ws = np.sin(wrep * svals).astype(np.float32)
    # u[p] = (qp2 + vb_sw)[p] * sinsw[p] must give +sin for rows p%64<32
    # (a_sin) and -sin for rows p%64>=32 (a_cos)
    sinsw = sin_ws.copy()
    sinsw[32:64] *= -1.0
    sinsw[96:128] *= -1.0

    # within-head swap of the 32-dim halves (for the rotation's sin term)
    sw_idx = np.arange(HL * HD)
    sw_idx = (sw_idx // HD) * HD + ((sw_idx % HD) + HALF) % HD
    sw128 = np.arange(128)
    sw128 = (sw128 // HD) * HD + ((sw128 % HD) + HALF) % HD
    pmat = np.zeros((128, 128), np.float32)
    pmat[sw128, np.arange(128)] = 1.0                    # P[p,i]=1 iff p=sw(i)

    WqT = Wq.T.astype(np.float32)                        # [D, D]
    WvT = Wv.T.astype(np.float32)
    WoT = Wo.T.astype(np.float32)                        # [Dv, D]

    qTs, vTs, kTs = [], [], []
    for b in range(B):
        qTs.append(_pack_cpn(query[b].T).astype(bf))     # [128, CT*S]
        vTs.append(_pack_cpn(value[b].T).astype(bf))
        kTs.append(np.ascontiguousarray(key[b].T))

    in_maps = []
    for c in range(NC_):
        b, g = c // 4, c % 4
        col0 = g * QW

        bql = bq[col0:col0 + QW]
        vbl = v_bias.reshape(-1)[col0:col0 + QW].astype(np.float32)
        WqTl = WqT[:, col0:col0 + QW]

        kp = kTs[b][col0:col0 + QW]                      # [256, S]
        kpp = np.ascontiguousarray(
            kp.reshape(HL, 64, S).transpose(1, 0, 2).reshape(64, HL * S))

        wop = _pack_cpn(WoT[col0:col0 + QW, :])          # [128, 2*D]

        vbl_sw = vbl[sw_idx]
        tabs = np.empty((128, 2 * S), np.float32)
        tabs[:, 0:S] = cos_ws
        tabs[:, S:2 * S] = sinsw
        vbcols = np.empty((128, 4), np.float32)
        for ft in range(2):
            vbcols[:, ft] = vbl[ft * 128:(ft + 1) * 128]
            vbcols[:, 2 + ft] = vbl_sw[ft * 128:(ft + 1) * 128]

        im = {
            "qw": _pack_cpn(WqTl).astype(bf),
            "qt": qTs[b],
            "wvp": _pack_cpn(WvT[:, col0:col0 + QW]).astype(bf),
            "vt": vTs[b],
            "kp": r(kpp),
            "g": g4,
            "wo": wop.astype(bf),
            "tabs": tabs.astype(bf),
            "vbc": vbcols,
            "pm": pmat,
        }
        if has_bq:
            im["wq9"] = bql[None, :].astype(bf)
        if has_bo:
            im["wo9"] = (bo / 4.0
                         + bv[col0:col0 + QW] @ WoT[col0:col0 + QW, :]
                         )[None, :].astype(bf)
        in_maps.append(im)
    return in_maps, has_bq, has_bo


def kernel(query, key, value, mask, Wq, bq, Wv, bv, Wo, bo, v_bias):
    from concourse.bass_utils import run_bass_kernel_spmd

    query = np.asarray(query, np.float32)
    key = np.asarray(key, np.float32)
    value = np.asarray(value, np.float32)
    in_maps, has_bq, has_bo = _host_pack(
        query, key, value,
        np.asarray(Wq, np.float32), np.asarray(bq, np.float32),
        np.asarray(Wv, np.float32), np.asarray(bv, np.float32),
        np.asarray(Wo, np.float32), np.asarray(bo, np.float32),
        np.asarray(v_bias, np.float32))

    ckey = ("nc", has_bq, has_bo)
    if ckey not in _cache:
        _cache[ckey] = _build_nc(has_bq, has_bo)
    nc = _cache[ckey]

    res = run_bass_kernel_spmd(
        nc, in_maps, core_ids=list(range(NC_)),
        trace=bool(int(os.environ.get("BASS_KERNEL_TRACE", "0"))))
    _cache["last_result"] = res

    out = np.empty((B, S, D), np.float32)
    for b in range(B):
        acc = res.results[4 * b]["out"].astype(np.float32)
        for g in range(1, 4):
            acc = acc + res.results[4 * b + g]["out"].astype(np.float32)
        out[b] = acc
    return out


# revision 50
# speedup vs baseline: 1.2352x; 1.1410x over previous
"""TENER-style MultiHeadedAttention TRN2 kernel (8 NeuronCores, SPMD).

Sharding (tensor-parallel over heads x data-parallel over batch):
core c handles batch b = c//4 and the 4 heads [4*(c%4), 4*(c%4)+4),
over ALL 1024 query rows. Wq/Wv are split column-wise by head, Wo
row-wise; each core emits a PARTIAL output [S, D] and the host gather
sums the 4 partials per batch (the Wo all-reduce).

Key math: the TENER relative-position term after the shift trick is
  rel[s, j] = (q_s + v_bias_h) . pos[j - s]
and pos rows are sinusoids, so by angle addition the whole score is ONE
128-deep contraction per head:
  scores[j, s] = [k_j ; sin(w j) ; cos(w j)] . [q_s ; a_sin(s) ; a_cos(s)]
  a_sin = qv_sin*cos(w s) + qv_cos*sin(w s)
  a_cos = qv_cos*cos(w s) - qv_sin*sin(w s)

All q/k-path matmuls run as float32r (full PE rate at free-dim >= 256).
The v projection runs in bf16 (halves its DMA traffic; v errors are not
exp-amplified). Softmax denominators come free via a ones-column per
head appended to v; normalization broadcasts the denominator row with a
rank-1 PE matmul and divides on DVE.
"""

import math
import os
import sys

sys.path.insert(0, "/opt/trn_rl_repo")

import numpy as np

B, S, D = 2, 1024, 1024
H, HD = 16, 64          # global heads, head_dim
HL = 4                  # local heads per core
HALF = 32               # sin/cos half of head_dim
NC_ = 8                 # cores
JT = S // 128           # 8 key tiles
CT = D // 128           # 8 contraction tiles

_cache: dict = {}


def _rne_fp32r(a):
    """Round fp32 -> fp32r (1s+8e+11m) with round-to-nearest-even."""
    u = np.ascontiguousarray(a, dtype=np.float32).view(np.uint32)
    lsb = (u >> np.uint32(12)) & np.uint32(1)
    return ((u + np.uint32(0x7FF) + lsb) & np.uint32(0xFFFFF000)).view(np.float32)


def _build_nc(has_bq: bool, has_bo: bool):
    import concourse.bacc as bacc
    import concourse.mybir as mybir
    from concourse import tile

    F32 = mybir.dt.float32
    F32R = mybir.dt.float32r
    BF16 = mybir.dt.bfloat16
    ADD = mybir.AluOpType.add
    MUL = mybir.AluOpType.mult
    EXP = mybir.ActivationFunctionType.Exp

    nc = bacc.Bacc("TRN2", target_bir_lowering=False, debug=False, num_devices=NC_)

    QW = HL * HD                      # 256 local q / v feature cols
    worows = QW + 1 if has_bo else QW
    qw_d = nc.dram_tensor("qw", [D + 1, 2 * QW], BF16, kind="ExternalInput")
    qt_d = nc.dram_tensor("qt", [D, S], BF16, kind="ExternalInput")
    wvp_d = nc.dram_tensor("wvp", [D, QW], BF16, kind="ExternalInput")
    vt_d = nc.dram_tensor("vt", [D, S], BF16, kind="ExternalInput")
    kp_d = nc.dram_tensor("kp", [HL * 64, S], F32R, kind="ExternalInput")
    g_d = nc.dram_tensor("g", [64, S], F32R, kind="ExternalInput")
    wo_d = nc.dram_tensor("wo", [worows, D], BF16, kind="ExternalInput")
    # tabs: [128, 1024 coss | 1024 sinsw | 2 vb-cols]
    tabs_d = nc.dram_tensor("tabs", [128, 2 * S + 2], F32, kind="ExternalInput")
    out_d = nc.dram_tensor("out", [S, D], BF16, kind="ExternalOutput")

    with tile.TileContext(nc, num_cores=NC_) as tc:
      with tc.tile_pool(name="persist", bufs=1) as pp, \
           tc.tile_pool(name="scratch", bufs=2) as sp, \
           tc.tile_pool(name="exppool", bufs=26) as ep, \
           tc.tile_pool(name="osb", bufs=4) as osb:

        # ---------- persistent SBUF ----------
        tabs = pp.tile([128, 2 * S + 2], F32, tag="tabs")
        wqt = pp.tile([128, CT * 2 * QW], BF16, tag="wqt")
        qtt = pp.tile([128, CT * S], BF16, tag="qtt")
        kgt = pp.tile([128, HL * S], F32R, tag="kgt")
        wvall = pp.tile([128, CT * QW], BF16, tag="wvall")
        vtall = pp.tile([128, CT * S], BF16, tag="vtall")
        wot = pp.tile([128, 2 * D], BF16, tag="wot")
        catq = [pp.tile([128, S], F32R, name=f"catq{h}", tag=f"catq{h}")
                for h in range(HL)]
        vv = [pp.tile([128, HL * (HD + 1)], BF16, name=f"vv{j}", tag=f"vv{j}")
              for j in range(JT)]
        xn = [pp.tile([128, S], BF16, name=f"xn{c}", tag=f"xn{c}")
              for c in range(2)]
        ebias = pp.tile([128, 1], F32, tag="ebias")
        wsb = pp.tile([128, 512], F32R, tag="wsb")
        wq9 = pp.tile([1, 2 * QW], BF16, tag="wq9")
        oq = pp.tile([1, S], BF16, tag="oq")
        if has_bo:
            wo9 = pp.tile([1, D], BF16, tag="wo9")
            xn1 = pp.tile([1, 128], BF16, tag="xn1")

        # ---------- input DMAs (priority waves per queue) ----------
        # sync: Wq | tabs | k | g | wv | Wo     gp: query half-tiles | vt
        nc.sync.dma_start(
            wqt[:].rearrange("p (c n) -> p c n", c=CT),
            qw_d.ap()[0:D, :].rearrange("(c p) n -> p c n", p=128))
        for half in range(2):
            for c in range(CT):
                nc.gpsimd.dma_start(
                    qtt[:, c * S + half * 512:c * S + half * 512 + 512],
                    qt_d.ap()[c * 128:(c + 1) * 128,
                              half * 512:half * 512 + 512])
        nc.sync.dma_start(tabs[:], tabs_d.ap())
        nc.sync.dma_start(wq9[:], qw_d.ap()[D:D + 1, :])
        nc.sync.dma_start(
            kgt[0:64, :].rearrange("p (h n) -> p h n", h=HL),
            kp_d.ap().rearrange("(h p) n -> p h n", p=64))
        nc.sync.dma_start(kgt[64:128, 0:S], g_d.ap())
        for h in range(1, HL):
            nc.sync.dma_start(kgt[64:128, h * S:(h + 1) * S],
                              kgt[64:128, 0:S])
        nc.sync.dma_start(
            wvall[:].rearrange("p (c n) -> p c n", c=CT),
            wvp_d.ap().rearrange("(c p) n -> p c n", p=128))
        nc.gpsimd.dma_start(
            vtall[:].rearrange("p (c n) -> p c n", c=CT),
            vt_d.ap().rearrange("(c p) n -> p c n", p=128))
        nc.sync.dma_start(
            wot[:].rearrange("p (c n) -> p c n", c=2),
            wo_d.ap()[0:QW, :].rearrange("(c p) n -> p c n", p=128))
        if has_bo:
            nc.sync.dma_start(wo9[:], wo_d.ap()[QW:QW + 1, :])

        # views
        kg = [kgt[:, h * S:(h + 1) * S] for h in range(HL)]
        wv = [wvall[:, c * QW:(c + 1) * QW] for c in range(CT)]
        vt = [vtall[:, c * S:(c + 1) * S] for c in range(CT)]
        wo = [wot[:, c * D:(c + 1) * D] for c in range(2)]
        coss = tabs[:, 0:S]
        sinsw = tabs[:, S:2 * S]

        # ---------- small inits ----------
        nc.vector.memset(ebias[:], -25.0)
        nc.vector.memset(wsb[:].bitcast(F32), 0.01)
        one_pair = float(np.array([0x3F803F80], np.uint32).view(np.float32)[0])
        nc.vector.memset(oq[:].bitcast(F32), one_pair)
        if has_bo:
            nc.vector.memset(xn1[:].bitcast(F32), one_pair)
        # fill vv with bf16 1.0s (the float below is two bf16 1.0s); the
        # vproj evictions overwrite the v columns, leaving the ones columns
        for j in range(JT):
            nc.vector.memset(vv[j][:].bitcast(F32), one_pair)

        # PSUM pools open/close in LIFO phase order within the 8-bank
        # budget: [sc 4 + qp 2] -> vp 2 -> xt 4.
        scps = tc.alloc_tile_pool(name="scps", bufs=2, space="PSUM")
        qps = tc.alloc_tile_pool(name="qps", bufs=1, space="PSUM")

        # ---------- q projection + rotation, per column half ----------
        # qp  = Wq_loc.T @ query.T           (raw q, feeds catq[0:64])
        # qp2 = Wq_sw.T @ query.T + vb_sw    (head-dim-swapped, feeds the
        #                                     sin term of the rotation)
        def qproj_half(ft, qp, qp2, half):
            hs = slice(half * 512, half * 512 + 512)
            for c in range(CT):
                nc.tensor.matmul(
                    qp[:, hs],
                    wqt[:, 2 * c * QW + ft * 128:2 * c * QW + (ft + 1) * 128],
                    qtt[:, c * S + half * 512:c * S + half * 512 + 512],
                    start=(c == 0), stop=(c == CT - 1 and not has_bq),
                    skip_group_check=True)
            if has_bq:
                nc.tensor.matmul(
                    qp[:, hs], wq9[:, ft * 128:(ft + 1) * 128],
                    oq[:, hs], start=False, stop=True, skip_group_check=True)
            for c in range(CT):
                nc.tensor.matmul(
                    qp2[:, hs],
                    wqt[:, (2 * c + 1) * QW + ft * 128:
                        (2 * c + 1) * QW + (ft + 1) * 128],
                    qtt[:, c * S + half * 512:c * S + half * 512 + 512],
                    start=(c == 0), stop=False, skip_group_check=True)
            nc.tensor.matmul(
                qp2[:, hs], wq9[:, QW + ft * 128:QW + (ft + 1) * 128],
                oq[:, hs], start=False, stop=True, skip_group_check=True)

        def rot_half(ft, qp, qp2, half):
            hs = slice(half * 512, half * 512 + 512)
            # q rows -> catq[0:64] (ACT partition-shift copies)
            nc.scalar.copy(catq[2 * ft][0:64, hs], qp[0:64, hs])
            nc.scalar.copy(catq[2 * ft + 1][0:64, hs], qp[64:128, hs])
            # rotation -> catq rows 64:128:
            #   t1 = (qp + vb) * cos(w s);  u = qp2 * sinsw
            #   catq[64:128] = t1 + u   (sin sign folded into sinsw)
            t1 = sp.tile([128, 512], F32, tag="t1")
            nc.vector.scalar_tensor_tensor(
                out=t1[:], in0=qp[:, hs],
                scalar=tabs[:, 2 * S + ft:2 * S + ft + 1],
                in1=coss[:, hs], op0=ADD, op1=MUL)
            u_ = sp.tile([128, 512], F32, tag="u_")
            nc.vector.tensor_tensor(out=u_[:], in0=qp2[:, hs],
                                    in1=sinsw[:, hs], op=MUL)
            for par in range(2):
                hq = 2 * ft + par
                o_ = par * 64
                nc.vector.tensor_tensor(
                    out=catq[hq][64:128, hs], in0=t1[o_:o_ + 64, :],
                    in1=u_[o_:o_ + 64, :], op=ADD)

        def qproj_rot(ft, qp, qp2):
            for half in range(2):
                qproj_half(ft, qp, qp2, half)
                rot_half(ft, qp, qp2, half)

        def scores_exp(h, jt_):
            sc = scps.tile([128, S], F32, tag="sc")
            for half in range(2):
                hs = slice(half * 512, half * 512 + 512)
                nc.tensor.matmul(
                    sc[:, hs], kg[h][:, jt_ * 128:(jt_ + 1) * 128],
                    catq[h][:, hs], start=True, stop=True,
                    skip_group_check=True)
            ex = ep.tile([128, S], BF16, tag="ex")
            nc.scalar.activation(ex[:], sc[:], EXP, bias=ebias[:], scale=1.0)
            return ex

        def attnv(h, jt_, ex, xt):
            for half in range(2):
                hs = slice(half * 512, half * 512 + 512)
                nc.tensor.matmul(
                    xt[0:HD + 1, hs],
                    vv[jt_][:, h * (HD + 1):(h + 1) * (HD + 1)], ex[:, hs],
                    start=(jt_ == 0), stop=(jt_ == JT - 1),
                    skip_group_check=True)

        def normalize(h, xt):
            # per q-half: reciprocal of the denom row, partition-broadcast
            # into SBUF, multiply (xt is the only PSUM operand)
            dsb = sp.tile([1, S], F32, tag="dsb")
            rsb = sp.tile([1, S], F32, tag="rsb")
            rbs = sp.tile([64, S], F32, tag="rbs")
            for half in range(2):
                hs = slice(half * 512, half * 512 + 512)
                nc.vector.tensor_copy(dsb[0:1, hs], xt[HD:HD + 1, hs])
                nc.vector.reciprocal_approx_fast(out=rsb[0:1, hs],
                                                 in_=dsb[0:1, hs])
                nc.gpsimd.partition_broadcast(rbs[:, hs], rsb[0:1, hs])
                nc.vector.tensor_tensor(
                    out=xn[h // 2][(h % 2) * 64:(h % 2) * 64 + 64, hs],
                    in0=xt[0:HD, hs], in1=rbs[:, hs], op=MUL)

        # ---------- emission: keep ACT (exp) busy end-to-end ----------
        # pre-warm: full-array dummies into the qp buffer before its first
        # real use, so the HAM clock gate is released before qproj starts
        wtile = qps.tile([128, S], F32, tag="qp")
        for _ in range(14):
            nc.tensor.matmul(wtile[0:128, 0:512], wsb[:, 0:128], wsb[:, :],
                             start=True, stop=True, skip_group_check=True)
        qp0 = qps.tile([128, S], F32, tag="qp")
        qp20 = qps.tile([128, S], F32, tag="qp2")
        qproj_rot(0, qp0, qp20)
        exh0 = [scores_exp(0, jt_) for jt_ in range(JT)]

        qp1 = qps.tile([128, S], F32, tag="qp")
        qp21 = qps.tile([128, S], F32, tag="qp2")
        qproj_rot(1, qp1, qp21)
        qps.release()

        exh1 = [scores_exp(1, jt_) for jt_ in range(JT)]
        exh2 = [scores_exp(2, jt_) for jt_ in range(JT)]

        # v projection interleaved with head-0 attnv consumption
        vps = tc.alloc_tile_pool(name="vps", bufs=2, space="PSUM")
        for jt_ in range(JT):
            vp = vps.tile([128, QW], F32, tag="vp")
            for c in range(CT):
                nc.tensor.matmul(
                    vp[:], vt[c][:, jt_ * 128:(jt_ + 1) * 128], wv[c][:],
                    start=(c == 0), stop=(c == CT - 1),
                    skip_group_check=True)
            dst = vv[jt_][:].rearrange("p (h x) -> p h x", x=HD + 1)[:, :, 0:HD]
            src_ = vp[:].rearrange("p (h d) -> p h d", d=HD)
            nc.vector.tensor_copy(dst, src_)
        vps.release()

        xtps = tc.alloc_tile_pool(name="xtps", bufs=2, space="PSUM")
        xt0 = xtps.tile([128, S], F32, tag="xt")
        xt1 = xtps.tile([128, S], F32, tag="xt")
        for jt_ in range(JT):
            attnv(0, jt_, exh0[jt_], xt0)
            attnv(1, jt_, exh1[jt_], xt1)
        normalize(0, xt0)
        normalize(1, xt1)

        xt2 = xtps.tile([128, S], F32, tag="xt")
        xt3 = xtps.tile([128, S], F32, tag="xt")
        for jt_ in range(JT):
            ex3 = scores_exp(3, jt_)
            attnv(2, jt_, exh2[jt_], xt2)
            attnv(3, jt_, ex3, xt3)
        normalize(2, xt2)
        normalize(3, xt3)

        # bridge the normalize-tail gap so the PE stays at full clock into
        # the output projection
        dtile = scps.tile([128, S], F32, tag="sc")
        for _ in range(16):
            nc.tensor.matmul(dtile[0:128, 0:512], wsb[:, 0:128], wsb[:, :],
                             start=True, stop=True, skip_group_check=True)
        xtps.release()
        scps.release()

        # ---------- output projection (partial out, bf16) ----------
        ops = tc.alloc_tile_pool(name="ops", bufs=3, space="PSUM")
        dengs = [nc.sync, nc.gpsimd, nc.scalar]
        for qt in range(8):
            op = ops.tile([128, D], F32, tag="op")
            for half in range(2):
                hs = slice(half * 512, half * 512 + 512)
                for c in range(2):
                    nc.tensor.matmul(
                        op[:, hs], xn[c][:, qt * 128:(qt + 1) * 128],
                        wo[c][:, hs], start=(c == 0),
                        stop=(c == 1 and not has_bo), skip_group_check=True)
                if has_bo:
                    nc.tensor.matmul(
                        op[:, hs], xn1[:], wo9[:, hs],
                        start=False, stop=True, skip_group_check=True)
            os_ = osb.tile([128, D], BF16, tag="os")
            # split the eviction across ACT + DVE so the PSUM bank frees at
            # ~2x single-engine copy rate; rotate the out DMA over all three
            # queues so the store stream never serializes on one queue
            nc.scalar.copy(os_[:, 0:512], op[:, 0:512])
            nc.vector.tensor_copy(os_[:, 512:1024], op[:, 512:1024])
            dengs[qt % 3].dma_start(out_d.ap()[qt * 128:(qt + 1) * 128, :],
                                    os_[:])
        ops.release()

    nc.finalize()
    return nc


def _host_pack(query, key, value, Wq, bq, Wv, bv, Wo, bo, v_bias):
    """Build the 8 per-core input maps."""
    import ml_dtypes
    r = _rne_fp32r
    bf = ml_dtypes.bfloat16
    QW = HL * HD
    w = np.exp(np.arange(HALF) * (-math.log(10000.0) / (HALF - 1)))

    has_bq = bool(np.any(bq))
    has_bo = bool(np.any(bo)) or bool(np.any(bv))

    # tables shared across the 4 cores of a batch except vb cols
    j = np.arange(S, dtype=np.float64)
    ang_j = w[:, None] * j[None, :]                      # [32, S]
    g64 = np.concatenate([np.sin(ang_j), np.cos(ang_j)], axis=0).astype(np.float32)

    wrep = np.tile(w, 4)[:, None]                        # [128, 1]
    svals = np.arange(S, dtype=np.float64)[None, :]
    cos_ws = np.cos(wrep * svals).astype(np.float32)     # [128, S]
    sin_ws = np.sin(wrep * svals).astype(np.float32)
    # u[p] = qp2[p] * sinsw[p] must give +sin for rows p%64<32 (a_sin) and
    # -sin for rows p%64>=32 (a_cos)
    sinsw = sin_ws.copy()
    sinsw[32:64] *= -1.0
    sinsw[96:128] *= -1.0

    # within-head swap of the 32-dim halves (for the rotation's sin term)
    sw_idx = np.arange(HL * HD)
    sw_idx = (sw_idx // HD) * HD + ((sw_idx % HD) + HALF) % HD

    WqT = Wq.T.astype(np.float32)                        # [D, D]
    WvT = Wv.T.astype(np.float32)
    WoT = Wo.T.astype(np.float32)                        # [Dv, D]

    g64r = r(g64)
    qTs, kTs, vTs = [], [], []
    for b in range(B):
        qTs.append(query[b].T.astype(bf))
        kTs.append(np.ascontiguousarray(key[b].T))
        vTs.append(value[b].T.astype(bf))

    in_maps = []
    for c in range(NC_):
        b, g = c // 4, c % 4
        col0 = g * QW

        bql = bq[col0:col0 + QW]
        vbl = v_bias.reshape(-1)[col0:col0 + QW].astype(np.float32)
        WqTl = WqT[:, col0:col0 + QW]
        qw = np.empty((D + 1, 2 * QW), np.float32)
        qw[:D, :QW] = WqTl
        qw[:D, QW:] = WqTl[:, sw_idx]
        qw[D, :QW] = bql
        qw[D, QW:] = (bql + vbl)[sw_idx]

        kp = np.ascontiguousarray(kTs[b][col0:col0 + QW])

        worows = QW + 1 if has_bo else QW
        wop = np.empty((worows, D), np.float32)
        wop[:QW] = WoT[col0:col0 + QW, :]
        if has_bo:
            wop[QW] = bo / 4.0 + bv[col0:col0 + QW] @ WoT[col0:col0 + QW, :]

        tabs = np.empty((128, 2 * S + 2), np.float32)
        tabs[:, 0:S] = cos_ws
        tabs[:, S:2 * S] = sinsw
        for ft in range(2):
            tabs[:, 2 * S + ft] = vbl[ft * 128:(ft + 1) * 128]

        in_maps.append({
            "qw": qw.astype(bf),
            "qt": qTs[b],
            "wvp": WvT[:, col0:col0 + QW].astype(bf),
            "vt": vTs[b],
            "kp": r(kp),
            "g": g64r,
            "wo": wop.astype(bf),
            "tabs": tabs,
        })
    return in_maps, has_bq, has_bo


def kernel(query, key, value, mask, Wq, bq, Wv, bv, Wo, bo, v_bias):
    from concourse.bass_utils import run_bass_kernel_spmd

    query = np.asarray(query, np.float32)
    key = np.asarray(key, np.float32)
    value = np.asarray(value, np.float32)
    in_maps, has_bq, has_bo = _host_pack(
        query, key, value,
        np.asarray(Wq, np.float32), np.asarray(bq, np.float32),
        np.asarray(Wv, np.float32), np.asarray(bv, np.float32),
        np.asarray(Wo, np.float32), np.asarray(bo, np.float32),
        np.asarray(v_bias, np.float32))

    ckey = ("nc", has_bq, has_bo)
    if ckey not in _cache:
        _cache[ckey] = _build_nc(has_bq, has_bo)
    nc = _cache[ckey]

    res = run_bass_kernel_spmd(
        nc, in_maps, core_ids=list(range(NC_)),
        trace=bool(int(os.environ.get("BASS_KERNEL_TRACE", "0"))))
    _cache["last_result"] = res

    out = np.empty((B, S, D), np.float32)
    for b in range(B):
        acc = res.results[4 * b]["out"].astype(np.float32)
        for g in range(1, 4):
            acc = acc + res.results[4 * b + g]["out"].astype(np.float32)
        out[b] = acc
    return out



# revision 54
# speedup vs baseline: 1.2620x; 1.0217x over previous
"""TENER-style MultiHeadedAttention TRN2 kernel (8 NeuronCores, SPMD).

Sharding (tensor-parallel over heads x data-parallel over batch):
core c handles batch b = c//4 and the 4 heads [4*(c%4), 4*(c%4)+4),
over ALL 1024 query rows. Wq/Wv are split column-wise by head, Wo
row-wise; each core emits a PARTIAL output [S, D] and the host gather
sums the 4 partials per batch (the Wo all-reduce).

Key math: the TENER relative-position term after the shift trick is
  rel[s, j] = (q_s + v_bias_h) . pos[j - s]
and pos rows are sinusoids, so by angle addition the whole score is ONE
128-deep contraction per head:
  scores[j, s] = [k_j ; sin(w j) ; cos(w j)] . [q_s ; a_sin(s) ; a_cos(s)]
  a_sin = qv_sin*cos(w s) + qv_cos*sin(w s)
  a_cos = qv_cos*cos(w s) - qv_sin*sin(w s)

All q/k-path matmuls run as float32r (full PE rate at free-dim >= 256).
The v projection runs in bf16 (halves its DMA traffic; v errors are not
exp-amplified). Softmax denominators come free via a ones-column per
head appended to v; normalization broadcasts the denominator row with a
rank-1 PE matmul and divides on DVE.
"""

import math
import os
import sys

sys.path.insert(0, "/opt/trn_rl_repo")

import numpy as np

B, S, D = 2, 1024, 1024
H, HD = 16, 64          # global heads, head_dim
HL = 4                  # local heads per core
HALF = 32               # sin/cos half of head_dim
NC_ = 8                 # cores
JT = S // 128           # 8 key tiles
CT = D // 128           # 8 contraction tiles

_cache: dict = {}


def _rne_fp32r(a):
    """Round fp32 -> fp32r (1s+8e+11m) with round-to-nearest-even."""
    u = np.ascontiguousarray(a, dtype=np.float32).view(np.uint32)
    lsb = (u >> np.uint32(12)) & np.uint32(1)
    return ((u + np.uint32(0x7FF) + lsb) & np.uint32(0xFFFFF000)).view(np.float32)


def _build_nc(has_bq: bool, has_bo: bool):
    import concourse.bacc as bacc
    import concourse.mybir as mybir
    from concourse import tile

    F32 = mybir.dt.float32
    F32R = mybir.dt.float32r
    BF16 = mybir.dt.bfloat16
    ADD = mybir.AluOpType.add
    MUL = mybir.AluOpType.mult
    EXP = mybir.ActivationFunctionType.Exp

    nc = bacc.Bacc("TRN2", target_bir_lowering=False, debug=False, num_devices=NC_)

    QW = HL * HD                      # 256 local q / v feature cols
    worows = QW + 1 if has_bo else QW
    qw_d = nc.dram_tensor("qw", [D + 1, 2 * QW], BF16, kind="ExternalInput")
    qt_d = nc.dram_tensor("qt", [D, S], BF16, kind="ExternalInput")
    wvp_d = nc.dram_tensor("wvp", [D, QW], BF16, kind="ExternalInput")
    vt_d = nc.dram_tensor("vt", [D, S], BF16, kind="ExternalInput")
    kp_d = nc.dram_tensor("kp", [HL * 64, S], F32R, kind="ExternalInput")
    g_d = nc.dram_tensor("g", [64, S], F32R, kind="ExternalInput")
    wo_d = nc.dram_tensor("wo", [worows, D], BF16, kind="ExternalInput")
    # tabs: [128, 1024 coss | 1024 sinsw | 2 vb-cols]
    tabs_d = nc.dram_tensor("tabs", [128, 2 * S + 2], F32, kind="ExternalInput")
    out_d = nc.dram_tensor("out", [S, D], BF16, kind="ExternalOutput")

    with tile.TileContext(nc, num_cores=NC_) as tc:
      with tc.tile_pool(name="persist", bufs=1) as pp, \
           tc.tile_pool(name="scratch", bufs=2) as sp, \
           tc.tile_pool(name="exppool", bufs=26) as ep, \
           tc.tile_pool(name="osb", bufs=4) as osb:

        # ---------- persistent SBUF ----------
        tabs = pp.tile([128, 2 * S + 2], F32, tag="tabs")
        wqt = pp.tile([128, CT * 2 * QW], BF16, tag="wqt")
        qtt = pp.tile([128, CT * S], BF16, tag="qtt")
        kgt = pp.tile([128, HL * S], F32R, tag="kgt")
        wvall = pp.tile([128, CT * QW], BF16, tag="wvall")
        vtall = pp.tile([128, CT * S], BF16, tag="vtall")
        wot = pp.tile([128, 2 * D], BF16, tag="wot")
        catq = [pp.tile([128, S], F32R, name=f"catq{h}", tag=f"catq{h}")
                for h in range(HL)]
        vv = [pp.tile([128, HL * (HD + 1)], BF16, name=f"vv{j}", tag=f"vv{j}")
              for j in range(JT)]
        xn = [pp.tile([128, S], BF16, name=f"xn{c}", tag=f"xn{c}")
              for c in range(2)]
        ebias = pp.tile([128, 1], F32, tag="ebias")
        wsb = pp.tile([128, 512], F32R, tag="wsb")
        wq9 = pp.tile([1, 2 * QW], BF16, tag="wq9")
        oq = pp.tile([1, S], BF16, tag="oq")
        if has_bo:
            wo9 = pp.tile([1, D], BF16, tag="wo9")
            xn1 = pp.tile([1, 128], BF16, tag="xn1")

        # ---------- input DMAs (priority waves per queue) ----------
        # sync: Wq | tabs | k | g | wv | Wo     gp: query half-tiles | vt
        nc.sync.dma_start(
            wqt[:].rearrange("p (c n) -> p c n", c=CT),
            qw_d.ap()[0:D, :].rearrange("(c p) n -> p c n", p=128))
        for half in range(2):
            for c in range(CT):
                nc.gpsimd.dma_start(
                    qtt[:, c * S + half * 512:c * S + half * 512 + 512],
                    qt_d.ap()[c * 128:(c + 1) * 128,
                              half * 512:half * 512 + 512])
        nc.sync.dma_start(tabs[:], tabs_d.ap())
        nc.sync.dma_start(wq9[:], qw_d.ap()[D:D + 1, :])
        nc.sync.dma_start(
            kgt[0:64, :].rearrange("p (h n) -> p h n", h=HL),
            kp_d.ap().rearrange("(h p) n -> p h n", p=64))
        nc.sync.dma_start(kgt[64:128, 0:S], g_d.ap())
        for h in range(1, HL):
            nc.sync.dma_start(kgt[64:128, h * S:(h + 1) * S],
                              kgt[64:128, 0:S])
        nc.sync.dma_start(
            wvall[:].rearrange("p (c n) -> p c n", c=CT),
            wvp_d.ap().rearrange("(c p) n -> p c n", p=128))
        nc.gpsimd.dma_start(
            vtall[:].rearrange("p (c n) -> p c n", c=CT),
            vt_d.ap().rearrange("(c p) n -> p c n", p=128))
        nc.sync.dma_start(
            wot[:].rearrange("p (c n) -> p c n", c=2),
            wo_d.ap()[0:QW, :].rearrange("(c p) n -> p c n", p=128))
        if has_bo:
            nc.sync.dma_start(wo9[:], wo_d.ap()[QW:QW + 1, :])

        # views
        kg = [kgt[:, h * S:(h + 1) * S] for h in range(HL)]
        wv = [wvall[:, c * QW:(c + 1) * QW] for c in range(CT)]
        vt = [vtall[:, c * S:(c + 1) * S] for c in range(CT)]
        wo = [wot[:, c * D:(c + 1) * D] for c in range(2)]
        coss = tabs[:, 0:S]
        sinsw = tabs[:, S:2 * S]

        # ---------- small inits ----------
        nc.vector.memset(ebias[:], -25.0)
        nc.vector.memset(wsb[:].bitcast(F32), 0.01)
        one_pair = float(np.array([0x3F803F80], np.uint32).view(np.float32)[0])
        nc.vector.memset(oq[:].bitcast(F32), one_pair)
        if has_bo:
            nc.vector.memset(xn1[:].bitcast(F32), one_pair)
        # fill vv with bf16 1.0s (the float below is two bf16 1.0s); the
        # vproj evictions overwrite the v columns, leaving the ones columns
        for j in range(JT):
            nc.vector.memset(vv[j][:].bitcast(F32), one_pair)

        # PSUM pools open/close in LIFO phase order within the 8-bank
        # budget: [sc 4 + qp 2] -> vp 2 -> xt 4.
        scps = tc.alloc_tile_pool(name="scps", bufs=2, space="PSUM")
        qps = tc.alloc_tile_pool(name="qps", bufs=1, space="PSUM")

        # ---------- q projection + rotation, per column half ----------
        # qp  = Wq_loc.T @ query.T           (raw q, feeds catq[0:64])
        # qp2 = Wq_sw.T @ query.T + vb_sw    (head-dim-swapped, feeds the
        #                                     sin term of the rotation)
        def qproj_half(ft, qp, qp2, half):
            hs = slice(half * 512, half * 512 + 512)
            for c in range(CT):
                nc.tensor.matmul(
                    qp[:, hs],
                    wqt[:, 2 * c * QW + ft * 128:2 * c * QW + (ft + 1) * 128],
                    qtt[:, c * S + half * 512:c * S + half * 512 + 512],
                    start=(c == 0), stop=(c == CT - 1 and not has_bq),
                    skip_group_check=True)
            if has_bq:
                nc.tensor.matmul(
                    qp[:, hs], wq9[:, ft * 128:(ft + 1) * 128],
                    oq[:, hs], start=False, stop=True, skip_group_check=True)
            for c in range(CT):
                nc.tensor.matmul(
                    qp2[:, hs],
                    wqt[:, (2 * c + 1) * QW + ft * 128:
                        (2 * c + 1) * QW + (ft + 1) * 128],
                    qtt[:, c * S + half * 512:c * S + half * 512 + 512],
                    start=(c == 0), stop=False, skip_group_check=True)
            nc.tensor.matmul(
                qp2[:, hs], wq9[:, QW + ft * 128:QW + (ft + 1) * 128],
                oq[:, hs], start=False, stop=True, skip_group_check=True)

        def rot_half(ft, qp, qp2, half):
            hs = slice(half * 512, half * 512 + 512)
            # q rows -> catq[0:64] (ACT partition-shift copies)
            nc.scalar.copy(catq[2 * ft][0:64, hs], qp[0:64, hs])
            nc.scalar.copy(catq[2 * ft + 1][0:64, hs], qp[64:128, hs])
            # rotation -> catq rows 64:128:
            #   t1 = (qp + vb) * cos(w s);  u = qp2 * sinsw
            #   catq[64:128] = t1 + u   (sin sign folded into sinsw)
            t1 = sp.tile([128, 512], F32, tag="t1")
            nc.vector.scalar_tensor_tensor(
                out=t1[:], in0=qp[:, hs],
                scalar=tabs[:, 2 * S + ft:2 * S + ft + 1],
                in1=coss[:, hs], op0=ADD, op1=MUL)
            u_ = sp.tile([128, 512], F32, tag="u_")
            nc.vector.tensor_tensor(out=u_[:], in0=qp2[:, hs],
                                    in1=sinsw[:, hs], op=MUL)
            for par in range(2):
                hq = 2 * ft + par
                o_ = par * 64
                nc.vector.tensor_tensor(
                    out=catq[hq][64:128, hs], in0=t1[o_:o_ + 64, :],
                    in1=u_[o_:o_ + 64, :], op=ADD)

        def qproj_rot(ft, qp, qp2):
            for half in range(2):
                qproj_half(ft, qp, qp2, half)
                rot_half(ft, qp, qp2, half)

        def scores_exp(h, jt_):
            sc = scps.tile([128, S], F32, tag="sc")
            for half in range(2):
                hs = slice(half * 512, half * 512 + 512)
                nc.tensor.matmul(
                    sc[:, hs], kg[h][:, jt_ * 128:(jt_ + 1) * 128],
                    catq[h][:, hs], start=True, stop=True,
                    skip_group_check=True)
            ex = ep.tile([128, S], BF16, tag="ex")
            nc.scalar.activation(ex[:], sc[:], EXP, bias=ebias[:], scale=1.0)
            return ex

        def attnv(h, jt_, ex, xt):
            for half in range(2):
                hs = slice(half * 512, half * 512 + 512)
                nc.tensor.matmul(
                    xt[0:HD + 1, hs],
                    vv[jt_][:, h * (HD + 1):(h + 1) * (HD + 1)], ex[:, hs],
                    start=(jt_ == 0), stop=(jt_ == JT - 1),
                    skip_group_check=True)

        def normalize_pair(pair):
            # software-pipelined across both heads: stage A (ACT stages the
            # denom row, DVE reciprocal, gpsimd broadcast) for both heads
            # first, then the multiplies — so the DVE queue never has head
            # A's multiply blocking head B's reciprocal, and the whole
            # chain drains ~2x faster after the last attnv matmul
            tiles = []
            for h, xt in pair:
                dsb = sp.tile([1, S], F32, tag="dsb")
                rsb = sp.tile([1, S], F32, tag="rsb")
                rbs = sp.tile([64, S], F32, tag="rbs")
                tiles.append((h, xt, dsb, rsb, rbs))
            for h, xt, dsb, rsb, rbs in tiles:
                for half in range(2):
                    hs = slice(half * 512, half * 512 + 512)
                    nc.scalar.copy(dsb[0:1, hs], xt[HD:HD + 1, hs])
                    nc.vector.reciprocal_approx_fast(out=rsb[0:1, hs],
                                                     in_=dsb[0:1, hs])
                    nc.gpsimd.partition_broadcast(rbs[:, hs], rsb[0:1, hs])
            for h, xt, dsb, rsb, rbs in tiles:
                nc.vector.tensor_tensor(
                    out=xn[h // 2][(h % 2) * 64:(h % 2) * 64 + 64, :],
                    in0=xt[0:HD, :], in1=rbs[:, :], op=MUL)

        # ---------- emission: keep ACT (exp) busy end-to-end ----------
        # pre-warm: full-array dummies into the qp buffer before its first
        # real use, so the HAM clock gate is released before qproj starts
        wtile = qps.tile([128, S], F32, tag="qp")
        for _ in range(14):
            nc.tensor.matmul(wtile[0:128, 0:512], wsb[:, 0:128], wsb[:, :],
                             start=True, stop=True, skip_group_check=True)
        qp0 = qps.tile([128, S], F32, tag="qp")
        qp20 = qps.tile([128, S], F32, tag="qp2")
        qproj_rot(0, qp0, qp20)
        exh0 = [scores_exp(0, jt_) for jt_ in range(JT)]

        qp1 = qps.tile([128, S], F32, tag="qp")
        qp21 = qps.tile([128, S], F32, tag="qp2")
        qproj_rot(1, qp1, qp21)
        qps.release()

        exh1 = [scores_exp(1, jt_) for jt_ in range(JT)]
        exh2 = [scores_exp(2, jt_) for jt_ in range(JT)]

        # v projection interleaved with head-0 attnv consumption
        vps = tc.alloc_tile_pool(name="vps", bufs=2, space="PSUM")
        for jt_ in range(JT):
            vp = vps.tile([128, QW], F32, tag="vp")
            for c in range(CT):
                nc.tensor.matmul(
                    vp[:], vt[c][:, jt_ * 128:(jt_ + 1) * 128], wv[c][:],
                    start=(c == 0), stop=(c == CT - 1),
                    skip_group_check=True)
            dst = vv[jt_][:].rearrange("p (h x) -> p h x", x=HD + 1)[:, :, 0:HD]
            src_ = vp[:].rearrange("p (h d) -> p h d", d=HD)
            nc.vector.tensor_copy(dst, src_)
        vps.release()

        xtps = tc.alloc_tile_pool(name="xtps", bufs=2, space="PSUM")
        xt0 = xtps.tile([128, S], F32, tag="xt")
        xt1 = xtps.tile([128, S], F32, tag="xt")
        for jt_ in range(JT):
            attnv(0, jt_, exh0[jt_], xt0)
            attnv(1, jt_, exh1[jt_], xt1)
        normalize_pair([(0, xt0), (1, xt1)])

        xt2 = xtps.tile([128, S], F32, tag="xt")
        xt3 = xtps.tile([128, S], F32, tag="xt")
        for jt_ in range(JT):
            ex3 = scores_exp(3, jt_)
            attnv(2, jt_, exh2[jt_], xt2)
            attnv(3, jt_, ex3, xt3)
        normalize_pair([(2, xt2), (3, xt3)])

        # bridge the normalize-tail gap so the PE stays at full clock into
        # the output projection
        dtile = scps.tile([128, S], F32, tag="sc")
        for _ in range(8):
            nc.tensor.matmul(dtile[0:128, 0:512], wsb[:, 0:128], wsb[:, :],
                             start=True, stop=True, skip_group_check=True)
        xtps.release()
        scps.release()

        # ---------- output projection (partial out, bf16) ----------
        ops = tc.alloc_tile_pool(name="ops", bufs=3, space="PSUM")
        dengs = [nc.sync, nc.gpsimd, nc.scalar]
        for qt in range(8):
            op = ops.tile([128, D], F32, tag="op")
            for half in range(2):
                hs = slice(half * 512, half * 512 + 512)
                for c in range(2):
                    nc.tensor.matmul(
                        op[:, hs], xn[c][:, qt * 128:(qt + 1) * 128],
                        wo[c][:, hs], start=(c == 0),
                        stop=(c == 1 and not has_bo), skip_group_check=True)
                if has_bo:
                    nc.tensor.matmul(
                        op[:, hs], xn1[:], wo9[:, hs],
                        start=False, stop=True, skip_group_check=True)
            os_ = osb.tile([128, D], BF16, tag="os")
            # split the eviction across ACT + DVE so the PSUM bank frees at
            # ~2x single-engine copy rate; rotate the out DMA over all three
            # queues so the store stream never serializes on one queue
            nc.scalar.copy(os_[:, 0:512], op[:, 0:512])
            nc.vector.tensor_copy(os_[:, 512:1024], op[:, 512:1024])
            dengs[qt % 3].dma_start(out_d.ap()[qt * 128:(qt + 1) * 128, :],
                                    os_[:])
        ops.release()

    nc.finalize()
    return nc


def _host_pack(query, key, value, Wq, bq, Wv, bv, Wo, bo, v_bias):
    """Build the 8 per-core input maps."""
    import ml_dtypes
    r = _rne_fp32r
    bf = ml_dtypes.bfloat16
    QW = HL * HD
    w = np.exp(np.arange(HALF) * (-math.log(10000.0) / (HALF - 1)))

    has_bq = bool(np.any(bq))
    has_bo = bool(np.any(bo)) or bool(np.any(bv))

    # tables shared across the 4 cores of a batch except vb cols
    j = np.arange(S, dtype=np.float64)
    ang_j = w[:, None] * j[None, :]                      # [32, S]
    g64 = np.concatenate([np.sin(ang_j), np.cos(ang_j)], axis=0).astype(np.float32)

    wrep = np.tile(w, 4)[:, None]                        # [128, 1]
    svals = np.arange(S, dtype=np.float64)[None, :]
    cos_ws = np.cos(wrep * svals).astype(np.float32)     # [128, S]
    sin_ws = np.sin(wrep * svals).astype(np.float32)
    # u[p] = qp2[p] * sinsw[p] must give +sin for rows p%64<32 (a_sin) and
    # -sin for rows p%64>=32 (a_cos)
    sinsw = sin_ws.copy()
    sinsw[32:64] *= -1.0
    sinsw[96:128] *= -1.0

    # within-head swap of the 32-dim halves (for the rotation's sin term)
    sw_idx = np.arange(HL * HD)
    sw_idx = (sw_idx // HD) * HD + ((sw_idx % HD) + HALF) % HD

    WqT = Wq.T.astype(np.float32)                        # [D, D]
    WvT = Wv.T.astype(np.float32)
    WoT = Wo.T.astype(np.float32)                        # [Dv, D]

    g64r = r(g64)
    qTs, kTs, vTs = [], [], []
    for b in range(B):
        qTs.append(query[b].T.astype(bf))
        kTs.append(np.ascontiguousarray(key[b].T))
        vTs.append(value[b].T.astype(bf))

    in_maps = []
    for c in range(NC_):
        b, g = c // 4, c % 4
        col0 = g * QW

        bql = bq[col0:col0 + QW]
        vbl = v_bias.reshape(-1)[col0:col0 + QW].astype(np.float32)
        WqTl = WqT[:, col0:col0 + QW]
        qw = np.empty((D + 1, 2 * QW), np.float32)
        qw[:D, :QW] = WqTl
        qw[:D, QW:] = WqTl[:, sw_idx]
        qw[D, :QW] = bql
        qw[D, QW:] = (bql + vbl)[sw_idx]

        kp = np.ascontiguousarray(kTs[b][col0:col0 + QW])

        worows = QW + 1 if has_bo else QW
        wop = np.empty((worows, D), np.float32)
        wop[:QW] = WoT[col0:col0 + QW, :]
        if has_bo:
            wop[QW] = bo / 4.0 + bv[col0:col0 + QW] @ WoT[col0:col0 + QW, :]

        tabs = np.empty((128, 2 * S + 2), np.float32)
        tabs[:, 0:S] = cos_ws
        tabs[:, S:2 * S] = sinsw
        for ft in range(2):
            tabs[:, 2 * S + ft] = vbl[ft * 128:(ft + 1) * 128]

        in_maps.append({
            "qw": qw.astype(bf),
            "qt": qTs[b],
            "wvp": WvT[:, col0:col0 + QW].astype(bf),
            "vt": vTs[b],
            "kp": r(kp),
            "g": g64r,
            "wo": wop.astype(bf),
            "tabs": tabs,
        })
    return in_maps, has_bq, has_bo


def kernel(query, key, value, mask, Wq, bq, Wv, bv, Wo, bo, v_bias):
    from concourse.bass_utils import run_bass_kernel_spmd

    query = np.asarray(query, np.float32)
    key = np.asarray(key, np.float32)
    value = np.asarray(value, np.float32)
    in_maps, has_bq, has_bo = _host_pack(
        query, key, value,
        np.asarray(Wq, np.float32), np.asarray(bq, np.float32),
        np.asarray(Wv, np.float32), np.asarray(bv, np.float32),
        np.asarray(Wo, np.float32), np.asarray(bo, np.float32),
        np.asarray(v_bias, np.float32))

    ckey = ("nc", has_bq, has_bo)
    if ckey not in _cache:
        _cache[ckey] = _build_nc(has_bq, has_bo)
    nc = _cache[ckey]

    res = run_bass_kernel_spmd(
        nc, in_maps, core_ids=list(range(NC_)),
        trace=bool(int(os.environ.get("BASS_KERNEL_TRACE", "0"))))
    _cache["last_result"] = res

    out = np.empty((B, S, D), np.float32)
    for b in range(B):
        acc = res.results[4 * b]["out"].astype(np.float32)
        for g in range(1, 4):
            acc = acc + res.results[4 * b + g]["out"].astype(np.float32)
        out[b] = acc
    return out



# revision 55
# speedup vs baseline: 1.2717x; 1.0076x over previous
"""TENER-style MultiHeadedAttention TRN2 kernel (8 NeuronCores, SPMD).

Sharding (tensor-parallel over heads x data-parallel over batch):
core c handles batch b = c//4 and the 4 heads [4*(c%4), 4*(c%4)+4),
over ALL 1024 query rows. Wq/Wv are split column-wise by head, Wo
row-wise; each core emits a PARTIAL output [S, D] and the host gather
sums the 4 partials per batch (the Wo all-reduce).

Key math: the TENER relative-position term after the shift trick is
  rel[s, j] = (q_s + v_bias_h) . pos[j - s]
and pos rows are sinusoids, so by angle addition the whole score is ONE
128-deep contraction per head:
  scores[j, s] = [k_j ; sin(w j) ; cos(w j)] . [q_s ; a_sin(s) ; a_cos(s)]
  a_sin = qv_sin*cos(w s) + qv_cos*sin(w s)
  a_cos = qv_cos*cos(w s) - qv_sin*sin(w s)

All q/k-path matmuls run as float32r (full PE rate at free-dim >= 256).
The v projection runs in bf16 (halves its DMA traffic; v errors are not
exp-amplified). Softmax denominators come free via a ones-column per
head appended to v; normalization broadcasts the denominator row with a
rank-1 PE matmul and divides on DVE.
"""

import math
import os
import sys

sys.path.insert(0, "/opt/trn_rl_repo")

import numpy as np

B, S, D = 2, 1024, 1024
H, HD = 16, 64          # global heads, head_dim
HL = 4                  # local heads per core
HALF = 32               # sin/cos half of head_dim
NC_ = 8                 # cores
JT = S // 128           # 8 key tiles
CT = D // 128           # 8 contraction tiles

_cache: dict = {}


def _rne_fp32r(a):
    """Round fp32 -> fp32r (1s+8e+11m) with round-to-nearest-even."""
    u = np.ascontiguousarray(a, dtype=np.float32).view(np.uint32)
    lsb = (u >> np.uint32(12)) & np.uint32(1)
    return ((u + np.uint32(0x7FF) + lsb) & np.uint32(0xFFFFF000)).view(np.float32)


def _build_nc(has_bq: bool, has_bo: bool):
    import concourse.bacc as bacc
    import concourse.mybir as mybir
    from concourse import tile

    F32 = mybir.dt.float32
    F32R = mybir.dt.float32r
    BF16 = mybir.dt.bfloat16
    ADD = mybir.AluOpType.add
    MUL = mybir.AluOpType.mult
    EXP = mybir.ActivationFunctionType.Exp

    nc = bacc.Bacc("TRN2", target_bir_lowering=False, debug=False, num_devices=NC_)

    QW = HL * HD                      # 256 local q / v feature cols
    worows = QW + 1 if has_bo else QW
    qw_d = nc.dram_tensor("qw", [D + 1, 2 * QW], BF16, kind="ExternalInput")
    qt_d = nc.dram_tensor("qt", [D, S], BF16, kind="ExternalInput")
    wvp_d = nc.dram_tensor("wvp", [D, QW], BF16, kind="ExternalInput")
    vt_d = nc.dram_tensor("vt", [D, S], BF16, kind="ExternalInput")
    kp_d = nc.dram_tensor("kp", [HL * 64, S], F32R, kind="ExternalInput")
    g_d = nc.dram_tensor("g", [64, S], F32R, kind="ExternalInput")
    wo_d = nc.dram_tensor("wo", [worows, D], BF16, kind="ExternalInput")
    # tabs: [128, 1024 coss | 1024 sinsw | 2 vb-cols]
    tabs_d = nc.dram_tensor("tabs", [128, 2 * S + 2], F32, kind="ExternalInput")
    out_d = nc.dram_tensor("out", [S, D], BF16, kind="ExternalOutput")

    with tile.TileContext(nc, num_cores=NC_) as tc:
      with tc.tile_pool(name="persist", bufs=1) as pp, \
           tc.tile_pool(name="scratch", bufs=2) as sp, \
           tc.tile_pool(name="exppool", bufs=26) as ep, \
           tc.tile_pool(name="osb", bufs=4) as osb:

        # ---------- persistent SBUF ----------
        tabs = pp.tile([128, 2 * S + 2], F32, tag="tabs")
        wqt = pp.tile([128, CT * 2 * QW], BF16, tag="wqt")
        qtt = pp.tile([128, CT * S], BF16, tag="qtt")
        kgt = pp.tile([128, HL * S], F32R, tag="kgt")
        wvall = pp.tile([128, CT * QW], BF16, tag="wvall")
        vtall = pp.tile([128, CT * S], BF16, tag="vtall")
        wot = pp.tile([128, 2 * D], BF16, tag="wot")
        catq = [pp.tile([128, S], F32R, name=f"catq{h}", tag=f"catq{h}")
                for h in range(HL)]
        vv = [pp.tile([128, HL * (HD + 1)], BF16, name=f"vv{j}", tag=f"vv{j}")
              for j in range(JT)]
        xn = [pp.tile([128, S], BF16, name=f"xn{c}", tag=f"xn{c}")
              for c in range(2)]
        ebias = pp.tile([128, 1], F32, tag="ebias")
        wsb = pp.tile([128, 512], F32R, tag="wsb")
        wq9 = pp.tile([1, 2 * QW], BF16, tag="wq9")
        oq = pp.tile([1, S], BF16, tag="oq")
        if has_bo:
            wo9 = pp.tile([1, D], BF16, tag="wo9")
            xn1 = pp.tile([1, 128], BF16, tag="xn1")

        # ---------- input DMAs (priority waves per queue) ----------
        # sync: Wq | tabs | k | g | wv | Wo     gp: query half-tiles | vt
        nc.sync.dma_start(
            wqt[:].rearrange("p (c n) -> p c n", c=CT),
            qw_d.ap()[0:D, :].rearrange("(c p) n -> p c n", p=128))
        for half in range(2):
            for c in range(CT):
                nc.gpsimd.dma_start(
                    qtt[:, c * S + half * 512:c * S + half * 512 + 512],
                    qt_d.ap()[c * 128:(c + 1) * 128,
                              half * 512:half * 512 + 512])
        nc.sync.dma_start(tabs[:], tabs_d.ap())
        nc.sync.dma_start(wq9[:], qw_d.ap()[D:D + 1, :])
        nc.sync.dma_start(
            kgt[0:64, :].rearrange("p (h n) -> p h n", h=HL),
            kp_d.ap().rearrange("(h p) n -> p h n", p=64))
        nc.sync.dma_start(kgt[64:128, 0:S], g_d.ap())
        for h in range(1, HL):
            nc.sync.dma_start(kgt[64:128, h * S:(h + 1) * S],
                              kgt[64:128, 0:S])
        nc.sync.dma_start(
            wvall[:].rearrange("p (c n) -> p c n", c=CT),
            wvp_d.ap().rearrange("(c p) n -> p c n", p=128))
        nc.gpsimd.dma_start(
            vtall[:].rearrange("p (c n) -> p c n", c=CT),
            vt_d.ap().rearrange("(c p) n -> p c n", p=128))
        nc.sync.dma_start(
            wot[:].rearrange("p (c n) -> p c n", c=2),
            wo_d.ap()[0:QW, :].rearrange("(c p) n -> p c n", p=128))
        if has_bo:
            nc.sync.dma_start(wo9[:], wo_d.ap()[QW:QW + 1, :])

        # views
        kg = [kgt[:, h * S:(h + 1) * S] for h in range(HL)]
        wv = [wvall[:, c * QW:(c + 1) * QW] for c in range(CT)]
        vt = [vtall[:, c * S:(c + 1) * S] for c in range(CT)]
        wo = [wot[:, c * D:(c + 1) * D] for c in range(2)]
        coss = tabs[:, 0:S]
        sinsw = tabs[:, S:2 * S]

        # ---------- small inits ----------
        nc.vector.memset(ebias[:], -25.0)
        nc.vector.memset(wsb[:].bitcast(F32), 0.01)
        one_pair = float(np.array([0x3F803F80], np.uint32).view(np.float32)[0])
        nc.vector.memset(oq[:].bitcast(F32), one_pair)
        if has_bo:
            nc.vector.memset(xn1[:].bitcast(F32), one_pair)
        # fill vv with bf16 1.0s (the float below is two bf16 1.0s); the
        # vproj evictions overwrite the v columns, leaving the ones columns
        for j in range(JT):
            nc.vector.memset(vv[j][:].bitcast(F32), one_pair)

        # PSUM pools open/close in LIFO phase order within the 8-bank
        # budget: [sc 4 + qp 2] -> vp 2 -> xt 4.
        scps = tc.alloc_tile_pool(name="scps", bufs=2, space="PSUM")
        qps = tc.alloc_tile_pool(name="qps", bufs=1, space="PSUM")

        # ---------- q projection + rotation, per column half ----------
        # qp  = Wq_loc.T @ query.T           (raw q, feeds catq[0:64])
        # qp2 = Wq_sw.T @ query.T + vb_sw    (head-dim-swapped, feeds the
        #                                     sin term of the rotation)
        def qproj_half(ft, qp, qp2, half):
            hs = slice(half * 512, half * 512 + 512)
            for c in range(CT):
                nc.tensor.matmul(
                    qp[:, hs],
                    wqt[:, 2 * c * QW + ft * 128:2 * c * QW + (ft + 1) * 128],
                    qtt[:, c * S + half * 512:c * S + half * 512 + 512],
                    start=(c == 0), stop=(c == CT - 1 and not has_bq),
                    skip_group_check=True)
            if has_bq:
                nc.tensor.matmul(
                    qp[:, hs], wq9[:, ft * 128:(ft + 1) * 128],
                    oq[:, hs], start=False, stop=True, skip_group_check=True)
            for c in range(CT):
                nc.tensor.matmul(
                    qp2[:, hs],
                    wqt[:, (2 * c + 1) * QW + ft * 128:
                        (2 * c + 1) * QW + (ft + 1) * 128],
                    qtt[:, c * S + half * 512:c * S + half * 512 + 512],
                    start=(c == 0), stop=False, skip_group_check=True)
            nc.tensor.matmul(
                qp2[:, hs], wq9[:, QW + ft * 128:QW + (ft + 1) * 128],
                oq[:, hs], start=False, stop=True, skip_group_check=True)

        def rot_half(ft, qp, qp2, half):
            hs = slice(half * 512, half * 512 + 512)
            # q rows -> catq[0:64] (ACT partition-shift copies)
            nc.scalar.copy(catq[2 * ft][0:64, hs], qp[0:64, hs])
            nc.scalar.copy(catq[2 * ft + 1][0:64, hs], qp[64:128, hs])
            # rotation -> catq rows 64:128:
            #   t1 = (qp + vb) * cos(w s);  u = qp2 * sinsw
            #   catq[64:128] = t1 + u   (sin sign folded into sinsw)
            t1 = sp.tile([128, 512], F32, tag="t1")
            nc.vector.scalar_tensor_tensor(
                out=t1[:], in0=qp[:, hs],
                scalar=tabs[:, 2 * S + ft:2 * S + ft + 1],
                in1=coss[:, hs], op0=ADD, op1=MUL)
            u_ = sp.tile([128, 512], F32, tag="u_")
            nc.vector.tensor_tensor(out=u_[:], in0=qp2[:, hs],
                                    in1=sinsw[:, hs], op=MUL)
            for par in range(2):
                hq = 2 * ft + par
                o_ = par * 64
                nc.vector.tensor_tensor(
                    out=catq[hq][64:128, hs], in0=t1[o_:o_ + 64, :],
                    in1=u_[o_:o_ + 64, :], op=ADD)

        def qproj_rot(ft, qp, qp2):
            for half in range(2):
                qproj_half(ft, qp, qp2, half)
                rot_half(ft, qp, qp2, half)

        def scores_exp(h, jt_):
            sc = scps.tile([128, S], F32, tag="sc")
            for half in range(2):
                hs = slice(half * 512, half * 512 + 512)
                nc.tensor.matmul(
                    sc[:, hs], kg[h][:, jt_ * 128:(jt_ + 1) * 128],
                    catq[h][:, hs], start=True, stop=True,
                    skip_group_check=True)
            ex = ep.tile([128, S], BF16, tag="ex")
            nc.scalar.activation(ex[:], sc[:], EXP, bias=ebias[:], scale=1.0)
            return ex

        def attnv(h, jt_, ex, xt):
            for half in range(2):
                hs = slice(half * 512, half * 512 + 512)
                nc.tensor.matmul(
                    xt[0:HD + 1, hs],
                    vv[jt_][:, h * (HD + 1):(h + 1) * (HD + 1)], ex[:, hs],
                    start=(jt_ == 0), stop=(jt_ == JT - 1),
                    skip_group_check=True)

        def normalize_pair(pair):
            # software-pipelined across both heads: stage A (ACT stages the
            # denom row, DVE reciprocal, gpsimd broadcast) for both heads
            # first, then the multiplies — so the DVE queue never has head
            # A's multiply blocking head B's reciprocal, and the whole
            # chain drains ~2x faster after the last attnv matmul
            tiles = []
            for h, xt in pair:
                dsb = sp.tile([1, S], F32, tag="dsb")
                rsb = sp.tile([1, S], F32, tag="rsb")
                rbs = sp.tile([64, S], F32, tag="rbs")
                tiles.append((h, xt, dsb, rsb, rbs))
            for h, xt, dsb, rsb, rbs in tiles:
                for half in range(2):
                    hs = slice(half * 512, half * 512 + 512)
                    nc.scalar.copy(dsb[0:1, hs], xt[HD:HD + 1, hs])
                    nc.vector.reciprocal_approx_fast(out=rsb[0:1, hs],
                                                     in_=dsb[0:1, hs])
                    nc.gpsimd.partition_broadcast(rbs[:, hs], rsb[0:1, hs])
            for h, xt, dsb, rsb, rbs in tiles:
                nc.vector.tensor_tensor(
                    out=xn[h // 2][(h % 2) * 64:(h % 2) * 64 + 64, :],
                    in0=xt[0:HD, :], in1=rbs[:, :], op=MUL)

        # ---------- emission: keep ACT (exp) busy end-to-end ----------
        # pre-warm: full-array dummies into the qp buffer before its first
        # real use, so the HAM clock gate is released before qproj starts
        wtile = qps.tile([128, S], F32, tag="qp")
        for _ in range(14):
            nc.tensor.matmul(wtile[0:128, 0:512], wsb[:, 0:128], wsb[:, :],
                             start=True, stop=True, skip_group_check=True)
        qp0 = qps.tile([128, S], F32, tag="qp")
        qp20 = qps.tile([128, S], F32, tag="qp2")
        qproj_rot(0, qp0, qp20)
        exh0 = [scores_exp(0, jt_) for jt_ in range(JT)]

        qp1 = qps.tile([128, S], F32, tag="qp")
        qp21 = qps.tile([128, S], F32, tag="qp2")
        qproj_rot(1, qp1, qp21)
        qps.release()

        exh1 = [scores_exp(1, jt_) for jt_ in range(JT)]
        exh2 = [scores_exp(2, jt_) for jt_ in range(JT)]

        # v projection interleaved with head-0 attnv consumption
        vps = tc.alloc_tile_pool(name="vps", bufs=2, space="PSUM")
        for jt_ in range(JT):
            vp = vps.tile([128, QW], F32, tag="vp")
            for c in range(CT):
                nc.tensor.matmul(
                    vp[:], vt[c][:, jt_ * 128:(jt_ + 1) * 128], wv[c][:],
                    start=(c == 0), stop=(c == CT - 1),
                    skip_group_check=True)
            dst = vv[jt_][:].rearrange("p (h x) -> p h x", x=HD + 1)[:, :, 0:HD]
            src_ = vp[:].rearrange("p (h d) -> p h d", d=HD)
            nc.vector.tensor_copy(dst, src_)
        vps.release()

        xtps = tc.alloc_tile_pool(name="xtps", bufs=2, space="PSUM")
        xt0 = xtps.tile([128, S], F32, tag="xt")
        xt1 = xtps.tile([128, S], F32, tag="xt")
        for jt_ in range(JT):
            attnv(0, jt_, exh0[jt_], xt0)
            attnv(1, jt_, exh1[jt_], xt1)
        normalize_pair([(0, xt0), (1, xt1)])

        xt2 = xtps.tile([128, S], F32, tag="xt")
        xt3 = xtps.tile([128, S], F32, tag="xt")
        for jt_ in range(JT):
            ex3 = scores_exp(3, jt_)
            attnv(2, jt_, exh2[jt_], xt2)
            attnv(3, jt_, ex3, xt3)
        normalize_pair([(2, xt2), (3, xt3)])

        # bridge the normalize-tail gap so the PE stays at full clock into
        # the output projection
        dtile = scps.tile([128, S], F32, tag="sc")
        for _ in range(12):
            nc.tensor.matmul(dtile[0:128, 0:512], wsb[:, 0:128], wsb[:, :],
                             start=True, stop=True, skip_group_check=True)
        xtps.release()
        scps.release()

        # ---------- output projection (partial out, bf16) ----------
        ops = tc.alloc_tile_pool(name="ops", bufs=3, space="PSUM")
        dengs = [nc.sync, nc.gpsimd, nc.scalar]
        for qt in range(8):
            op = ops.tile([128, D], F32, tag="op")
            for half in range(2):
                hs = slice(half * 512, half * 512 + 512)
                for c in range(2):
                    nc.tensor.matmul(
                        op[:, hs], xn[c][:, qt * 128:(qt + 1) * 128],
                        wo[c][:, hs], start=(c == 0),
                        stop=(c == 1 and not has_bo), skip_group_check=True)
                if has_bo:
                    nc.tensor.matmul(
                        op[:, hs], xn1[:], wo9[:, hs],
                        start=False, stop=True, skip_group_check=True)
            os_ = osb.tile([128, D], BF16, tag="os")
            # split the eviction across ACT + DVE so the PSUM bank frees at
            # ~2x single-engine copy rate; rotate the out DMA over all three
            # queues so the store stream never serializes on one queue
            nc.scalar.copy(os_[:, 0:512], op[:, 0:512])
            nc.vector.tensor_copy(os_[:, 512:1024], op[:, 512:1024])
            dengs[qt % 3].dma_start(out_d.ap()[qt * 128:(qt + 1) * 128, :],
                                    os_[:])
        ops.release()

    nc.finalize()
    return nc


def _host_pack(query, key, value, Wq, bq, Wv, bv, Wo, bo, v_bias):
    """Build the 8 per-core input maps."""
    import ml_dtypes
    r = _rne_fp32r
    bf = ml_dtypes.bfloat16
    QW = HL * HD
    w = np.exp(np.arange(HALF) * (-math.log(10000.0) / (HALF - 1)))

    has_bq = bool(np.any(bq))
    has_bo = bool(np.any(bo)) or bool(np.any(bv))

    # tables shared across the 4 cores of a batch except vb cols
    j = np.arange(S, dtype=np.float64)
    ang_j = w[:, None] * j[None, :]                      # [32, S]
    g64 = np.concatenate([np.sin(ang_j), np.cos(ang_j)], axis=0).astype(np.float32)

    wrep = np.tile(w, 4)[:, None]                        # [128, 1]
    svals = np.arange(S, dtype=np.float64)[None, :]
    cos_ws = np.cos(wrep * svals).astype(np.float32)     # [128, S]
    sin_ws = np.sin(wrep * svals).astype(np.float32)
    # u[p] = qp2[p] * sinsw[p] must give +sin for rows p%64<32 (a_sin) and
    # -sin for rows p%64>=32 (a_cos)
    sinsw = sin_ws.copy()
    sinsw[32:64] *= -1.0
    sinsw[96:128] *= -1.0

    # within-head swap of the 32-dim halves (for the rotation's sin term)
    sw_idx = np.arange(HL * HD)
    sw_idx = (sw_idx // HD) * HD + ((sw_idx % HD) + HALF) % HD

    WqT = Wq.T.astype(np.float32)                        # [D, D]
    WvT = Wv.T.astype(np.float32)
    WoT = Wo.T.astype(np.float32)                        # [Dv, D]

    g64r = r(g64)
    qTs, kTs, vTs = [], [], []
    for b in range(B):
        qTs.append(query[b].T.astype(bf))
        kTs.append(np.ascontiguousarray(key[b].T))
        vTs.append(value[b].T.astype(bf))

    in_maps = []
    for c in range(NC_):
        b, g = c // 4, c % 4
        col0 = g * QW

        bql = bq[col0:col0 + QW]
        vbl = v_bias.reshape(-1)[col0:col0 + QW].astype(np.float32)
        WqTl = WqT[:, col0:col0 + QW]
        qw = np.empty((D + 1, 2 * QW), np.float32)
        qw[:D, :QW] = WqTl
        qw[:D, QW:] = WqTl[:, sw_idx]
        qw[D, :QW] = bql
        qw[D, QW:] = (bql + vbl)[sw_idx]

        kp = np.ascontiguousarray(kTs[b][col0:col0 + QW])

        worows = QW + 1 if has_bo else QW
        wop = np.empty((worows, D), np.float32)
        wop[:QW] = WoT[col0:col0 + QW, :]
        if has_bo:
            wop[QW] = bo / 4.0 + bv[col0:col0 + QW] @ WoT[col0:col0 + QW, :]

        tabs = np.empty((128, 2 * S + 2), np.float32)
        tabs[:, 0:S] = cos_ws
        tabs[:, S:2 * S] = sinsw
        for ft in range(2):
            tabs[:, 2 * S + ft] = vbl[ft * 128:(ft + 1) * 128]

        in_maps.append({
            "qw": qw.astype(bf),
            "qt": qTs[b],
            "wvp": WvT[:, col0:col0 + QW].astype(bf),
            "vt": vTs[b],
            "kp": r(kp),
            "g": g64r,
            "wo": wop.astype(bf),
            "tabs": tabs,
        })
    return in_maps, has_bq, has_bo


def kernel(query, key, value, mask, Wq, bq, Wv, bv, Wo, bo, v_bias):
    from concourse.bass_utils import run_bass_kernel_spmd

    query = np.asarray(query, np.float32)
    key = np.asarray(key, np.float32)
    value = np.asarray(value, np.float32)
    in_maps, has_bq, has_bo = _host_pack(
        query, key, value,
        np.asarray(Wq, np.float32), np.asarray(bq, np.float32),
        np.asarray(Wv, np.float32), np.asarray(bv, np.float32),
        np.asarray(Wo, np.float32), np.asarray(bo, np.float32),
        np.asarray(v_bias, np.float32))

    ckey = ("nc", has_bq, has_bo)
    if ckey not in _cache:
        _cache[ckey] = _build_nc(has_bq, has_bo)
    nc = _cache[ckey]

    res = run_bass_kernel_spmd(
        nc, in_maps, core_ids=list(range(NC_)),
        trace=bool(int(os.environ.get("BASS_KERNEL_TRACE", "0"))))
    _cache["last_result"] = res

    out = np.empty((B, S, D), np.float32)
    for b in range(B):
        acc = res.results[4 * b]["out"].astype(np.float32)
        for g in range(1, 4):
            acc = acc + res.results[4 * b + g]["out"].astype(np.float32)
        out[b] = acc
    return out

